# revision 69
# baseline (speedup 1.0000x reference)
"""Trainium2 Bass kernel: 3D-RoPE multi-head attention (B=4,N=2048,DIM=1536,H=16,DH=96).

Sharding: 8 cores = (batch b = c//2) x (head-group g = c%2, 8 heads each).
Each core:
  - projects Q,K,V for its 8 heads over all 2048 tokens (tensor parallel:
    Wqkv column-split, Wout row-split). K/Q projected in PACKED 128-row
    tiles of the [768, N] K^T/Q^T matrix (full PE-array utilization), RoPE
    applied per packed tile (3 distinct 128x128 sign-permutations,
    host-precomputed replicated cos/sin), then DMA-unpacked into per-head
    [96, N] tiles.
  - attention per head (softmax over keys, no max-subtraction), query-half
    outer loop; output-projection tiles for the finished query half are
    interleaved into the next half's head loop to fill the PE slack left
    by the ACT-bound exp stream (outproj PSUM shares the ho pool ring).
  - normalized head outputs DMA-packed into [128, 6, N] tiles; partial
    output projection contracts 6x128 rows of Wout.
Host gather: out[b] = core(2b) + core(2b+1) partial sums (bias on g=0).
All matmul inputs bf16, fp32 PSUM accumulation, no DRAM spills.
"""

import sys

if "/opt/trn_rl_repo" not in sys.path:
    sys.path.insert(0, "/opt/trn_rl_repo")

import numpy as np

import concourse.bass as bass
import concourse.mybir as mybir
import concourse.tile as tile
from concourse import bacc
from concourse.bass_utils import run_bass_kernel_spmd

B, N, DIM, H, DH = 4, 2048, 1536, 16, 96
HG = 8            # heads per core
G = HG * DH       # 768 rows/cols per group
RT = G // 128     # 6 packed row tiles
KT = DIM // 128   # 12 contraction tiles
HN = N // 2       # token half (projection passes) / query chunk
NMT = N // 128    # 16 key tiles
SCALE = DH ** -0.5
F32 = mybir.dt.float32
F32R = mybir.dt.float32r
BF16 = mybir.dt.bfloat16
F16 = mybir.dt.float16
import ml_dtypes
IN_NP = ml_dtypes.bfloat16
AF = mybir.ActivationFunctionType

# packed-tile segment maps (128-row tile j of a [384,*] block, period 3)
# replica rows: row r of tile j = master row (128*j + r) % 96
REP_SEGS = {
    0: [(0, 96, 0), (96, 128, 0)],
    1: [(0, 64, 32), (64, 128, 0)],
    2: [(0, 32, 64), (32, 128, 0)],
}
# per packed tile rt: (head, tile_lo, tile_hi, head_row_lo)
def _tile_segs(rt):
    segs = []
    r = 128 * rt
    while r < 128 * rt + 128:
        h = r // 96
        end = min(128 * rt + 128, (h + 1) * 96)
        segs.append((h, r - 128 * rt, end - 128 * rt, r % 96))
        r = end
    return segs
TILE_SEGS = {rt: _tile_segs(rt) for rt in range(RT)}
# per head: list of (rt, tile_lo, tile_hi, head_row_lo)
HEAD_SEGS = {h: [] for h in range(HG)}
for rt in range(RT):
    for h, a, b, d0 in TILE_SEGS[rt]:
        HEAD_SEGS[h].append((rt, a, b, d0))


def _build_rope_pt_packed() -> np.ndarray:
    """lhsT tiles for rotate_half on packed 128-row layout: A[:, j*128:(j+1)*128]
    = P_j^T where rot_packed = P_j @ t_packed for tile j (j = rt % 3)."""
    A = np.zeros((128, 3 * 128), np.float32)
    for j in range(3):
        for r in range(128):
            Rg = 128 * j + r
            d = Rg % 96
            c, a = d // 32, d % 32
            sign = -1.0 if a < 16 else 1.0
            dq = 32 * c + (a + 16) % 32
            qg = (Rg // 96) * 96 + dq
            ql = qg - 128 * j
            # P_j[r, ql] = sign; lhsT[k, i] = P_j[i, k]
            A[ql, j * 128 + r] = sign
    return np.ascontiguousarray(A)


def _emit(ctx, tc, io):
    nc = tc.nc
    xbP, cosP, sinP, ropePT, WgT, WoT, bout, out = io

    def mm512(out_ap, lhsT, rhs, start, stop, width=HN):
        for c0 in range(0, width, 512):
            nc.tensor.matmul(
                out=out_ap[:, c0:c0 + 512], lhsT=lhsT,
                rhs=rhs[:, c0:c0 + 512], start=start, stop=stop,
            )

    persist = ctx.enter_context(tc.tile_pool(name="persist", bufs=1))

    # ---- constants ------------------------------------------------------
    ropeP_sb = persist.tile([128, 3, 128], BF16, tag="ropeP")
    nc.sync.dma_start(out=ropeP_sb, in_=ropePT.rearrange("p (j c) -> p j c", j=3))
    ones1f = persist.tile([1, DH], F32, tag="ones1f")
    nc.vector.memset(ones1f, 1.0)
    ones1 = persist.tile([1, DH], F32R, tag="ones1")
    nc.scalar.copy(out=ones1, in_=ones1f)

    # resident tensors
    KTr = persist.tile([DH, HG, N], BF16, tag="KTr")
    QTr = persist.tile([DH, HG, N], BF16, tag="QTr")
    Vr = persist.tile([128, NMT, HG, DH + 1], BF16, tag="Vr")
    nc.vector.memset(Vr[:, :, :, DH:DH + 1], 1.0)

    # ---- projections (two token-half passes, shared xb tiles) ------------
    # WgT columns: [V ct0 | V ct1 | K rt0..5 | Q rt0..5], each tile k-major
    # and per-partition contiguous. xbP rows: [p*128 + partition].
    with tc.tile_pool(name="wv", bufs=1) as pwv:
        # V weights are token-independent: both column tiles load once for
        # both passes; wv0 leads the scalar queue (first-matmul critical
        # path), wv1 follows p0's xb1
        wvt = [pwv.tile([128, KT, 384], BF16, tag=f"wv{ct}", name=f"wv{ct}")
               for ct in range(2)]
        nc.scalar.dma_start(out=wvt[0], in_=WgT[:, 0:4608])
        # host-precomputed replicated cos/sin for packed rope [128, 3, N];
        # projection-scoped so attention inherits the 24KB
        cosR = pwv.tile([128, 3, N], BF16, tag="cosR")
        sinR = pwv.tile([128, 3, N], BF16, tag="sinR")

        rope_pending = None

        def emit_rope(ps, dst, tok):
            """Packed rope on finished projection group ps -> dst per-head."""
            rt = ps_rt[id(ps)]
            j = rt % 3
            t_sb = prp.tile([128, HN], BF16, tag="rope_t", name="rope_t")
            nc.vector.tensor_copy(out=t_sb, in_=ps)
            rot = pskr.tile([128, HN], F32, tag="rope_rot", name="rope_rot")
            mm512(rot, ropeP_sb[:, j, :], t_sb, start=True, stop=True)
            u = prp.tile([128, HN], BF16, tag="rope_u", name="rope_u")
            nc.vector.tensor_mul(out=u, in0=t_sb, in1=cosR[:, j, tok])
            nc.vector.tensor_mul(out=rot, in0=rot, in1=sinR[:, j, tok])
            st = prp.tile([128, HN], BF16, tag="rope_st", name="rope_st")
            nc.vector.tensor_add(out=st, in0=u, in1=rot)
            for (h, a, b, d0) in TILE_SEGS[rt]:
                nc.sync.dma_start(
                    out=dst[d0:d0 + (b - a), h, tok], in_=st[a:b, :]
                )

        ps_rt = {}
        with (
            tc.tile_pool(name="xb", bufs=2) as pxb,
            tc.tile_pool(name="wkq", bufs=3) as pwkq,
            tc.tile_pool(name="vst", bufs=6) as pvst,
            tc.tile_pool(name="rope", bufs=2) as prp,
            tc.tile_pool(name="psk", bufs=2, space="PSUM") as psk,
            tc.tile_pool(name="pskr", bufs=1, space="PSUM") as pskr,
        ):
            for p in range(2):
                tok = slice(p * HN, (p + 1) * HN)
                xb3 = []
                xb_q = (nc.sync, nc.scalar, nc.sync)
                for i in range(3):
                    t3 = pxb.tile([128, 4, HN], BF16, tag=f"xb{i}",
                                  name=f"xb{p}_{i}")
                    xb_q[i].dma_start(
                        out=t3,
                        in_=xbP[p * 128:(p + 1) * 128, i * 4 * HN:(i + 1) * 4 * HN],
                    )
                    xb3.append(t3)
                if p == 0:
                    # wv1 after p0's xb1 on scalar (queue transfers
                    # serialize; first ct1 use is ~27us in)
                    nc.scalar.dma_start(out=wvt[1], in_=WgT[:, 4608:9216])
                xb = [xb3[k // 4][:, k % 4, :] for k in range(KT)]
                # V projection: [token, vcol] tiles, 2 col tiles of 384.
                # PSUM accumulation is order-independent: consume xb tiles
                # in DMA-arrival order (xb0 sync, xb2 sync, xb1 scalar) so
                # the first chains start before all of x has landed
                KORD = [0, 1, 2, 3, 8, 9, 10, 11, 4, 5, 6, 7]
                for ct in range(2):
                    wv = wvt[ct]
                    for tt in range(HN // 128):
                        ps = psk.tile([128, 384], F32, tag="kps", name="vps")
                        for ki, k in enumerate(KORD):
                            nc.tensor.matmul(
                                out=ps, lhsT=xb[k][:, tt * 128:(tt + 1) * 128],
                                rhs=wv[:, k, :], start=(ki == 0),
                                stop=(ki == KT - 1),
                            )
                        vst = pvst.tile([128, 4, DH], BF16, tag="vst", name="vst")
                        nc.vector.tensor_copy(out=vst, in_=ps)
                        mt = p * (HN // 128) + tt
                        # alternate staging queues to halve queue latency
                        (nc.sync if tt % 2 == 0 else nc.scalar).dma_start(
                            out=Vr[:, mt, ct * 4:(ct + 1) * 4, 0:DH], in_=vst
                        )
                # K and Q projection, packed 128-row tiles + lagged rope
                for dst, si, nm in ((KTr, 0, "k"), (QTr, 1, "q")):
                    for rt in range(RT):
                        cs = 9216 + si * 9216 + rt * 1536
                        wk = pwkq.tile([128, KT, 128], BF16, tag="wkq",
                                       name=f"w{nm}{p}_{rt}")
                        nc.scalar.dma_start(out=wk, in_=WgT[:, cs:cs + 1536])
                        if p == 0 and si == 0 and rt == 0:
                            # trig behind the first K weights: its transfer
                            # must not race the startup-critical loads
                            # (DMA bandwidth is shared across queues);
                            # first rope use is ~45us in
                            nc.scalar.dma_start(out=cosR, in_=cosP)
                            nc.scalar.dma_start(out=sinR, in_=sinP)
                        ps = psk.tile([128, HN], F32, tag="kps", name="kps")
                        ps_rt[id(ps)] = rt
                        for k in range(KT):
                            mm512(ps, wk[:, k, :], xb[k], start=(k == 0),
                                  stop=(k == KT - 1))
                        if rope_pending is not None:
                            emit_rope(*rope_pending)
                        rope_pending = (ps, dst, tok)
            emit_rope(*rope_pending)

    # ---- attention + packed ho + interleaved output projection -----------
    span = ctx.enter_context(tc.tile_pool(name="span", bufs=1))
    hoP = span.tile([128, RT, N], BF16, tag="hoP")
    bias_sb = span.tile([128, DIM], F32, tag="bias")
    bout_bc = bass.AP(tensor=bout.tensor, offset=bout.offset,
                      ap=[[0, 128]] + [list(p) for p in bout.ap])
    nc.sync.dma_start(out=bias_sb, in_=bout_bc)
    wo = []
    for et in range(DIM // 512):
        wt = span.tile([128, RT, 512], BF16, tag=f"wo{et}", name=f"wo{et}")
        nc.sync.dma_start(out=wt, in_=WoT[:, et, :])
        wo.append(wt)
    with (
        tc.tile_pool(name="ex", bufs=4) as pex,
        tc.tile_pool(name="hur", bufs=3) as phur,
        tc.tile_pool(name="stn", bufs=2) as pstn,
        tc.tile_pool(name="rcd", bufs=1) as prcd,
        tc.tile_pool(name="rc", bufs=2) as prc,
        tc.tile_pool(name="osb", bufs=4) as posb,
        tc.tile_pool(name="rbd", bufs=2, space="DRAM") as prbd,
        tc.tile_pool(name="bcs", bufs=2) as pbcs,
        tc.tile_pool(name="psho", bufs=2, space="PSUM") as psho,
        tc.tile_pool(name="pssc", bufs=3, space="PSUM") as pssc,
    ):
        def normalize(h, qt, rcf, hu, hops_bc=False):
            """stage = hu * broadcast(rc); DMA-pack staged rows into hoP.

            The broadcast runs in the eps ring (idle during attention) so it
            never shrinks the sc ring's pipeline depth; the final flush uses
            the freed ho slot instead (eps+scps hold open pre-tail groups).
            """
            qsl = slice(qt * HN, (qt + 1) * HN)
            stn = pstn.tile([DH, HN], BF16, tag="stn", name="stn")
            if hops_bc:
                # final flush: PE is idle then, and the matmul broadcast is
                # lower-latency than the DRAM bounce; final_rc was converted
                # to f32r eagerly inside the last head's chain
                bc = psho.tile([DH, HN], F32, tag="hops", name="bcps",
                               bufs=1)
                mm512(bc, ones1, final_rc, start=True, stop=True)
                nc.vector.tensor_mul(out=stn, in0=hu[0:DH, :], in1=bc)
            else:
                # steady state: broadcast via DRAM bounce (stride-0 reads
                # are legal from DRAM) — zero PE work, latency hidden by
                # the one-head normalize lag
                rb = prbd.tile([1, HN], F32, tag="rbd", name="rbd")
                nc.sync.dma_start(out=rb, in_=rcf)
                rb_bc = bass.AP(tensor=rb.tensor, offset=rb.offset,
                                ap=[[0, DH]] + [list(p) for p in rb.ap[1:]])
                bc = pbcs.tile([DH, HN], F32, tag="bcs", name="bcs")
                nc.sync.dma_start(out=bc, in_=rb_bc)
                nc.vector.tensor_mul(out=stn, in0=hu[0:DH, :], in1=bc)
            for (rt, a, b, d0) in HEAD_SEGS[h]:
                nc.sync.dma_start(
                    out=hoP[a:b, rt, qsl], in_=stn[d0:d0 + (b - a), :]
                )

        out_qs = [nc.sync, nc.scalar]

        def emit_outproj(et, tt):
            """One [128-token, 512-col] output tile; own PSUM ring."""
            ps = pssc.tile([128, 512], F32, tag="scps", name="eps")
            for c in range(RT):
                nc.tensor.matmul(
                    out=ps, lhsT=hoP[:, c, tt * 128:(tt + 1) * 128],
                    rhs=wo[et][:, c, :], start=(c == 0), stop=(c == RT - 1),
                )
            osb = posb.tile([128, 512], F32, tag="osb", name="osb")
            nc.vector.tensor_add(
                out=osb, in0=ps, in1=bias_sb[:, et * 512:(et + 1) * 512]
            )
            out_qs[(et + tt) % 2].dma_start(
                out=out[tt * 128:(tt + 1) * 128, et * 512:(et + 1) * 512],
                in_=osb,
            )

        outq = []
        # mt slot -> chunk phase for spread-out drains
        DRAIN_MT = {5: 0, 6: 1, 7: 2, 10: 0, 11: 1, 12: 2, 13: 0, 14: 1,
                    15: 2}
        cur_drain = [None]

        def drain_chunk(phase):
            if phase == 0:
                if not outq:
                    cur_drain[0] = None
                    return
                et, tt = outq.pop(0)
                ps = pssc.tile([128, 512], F32, tag="scps", name="eps")
                cur_drain[0] = (et, tt, ps)
            if cur_drain[0] is None:
                return
            et, tt, ps = cur_drain[0]
            for c in (2 * phase, 2 * phase + 1):
                nc.tensor.matmul(
                    out=ps, lhsT=hoP[:, c, tt * 128:(tt + 1) * 128],
                    rhs=wo[et][:, c, :], start=(c == 0), stop=(c == RT - 1),
                )
            if phase == 2:
                osb = posb.tile([128, 512], F32, tag="osb", name="osb")
                nc.vector.tensor_add(
                    out=osb, in0=ps, in1=bias_sb[:, et * 512:(et + 1) * 512]
                )
                out_qs[(et + tt) % 2].dma_start(
                    out=out[tt * 128:(tt + 1) * 128,
                            et * 512:(et + 1) * 512],
                    in_=osb,
                )
                cur_drain[0] = None

        pending = None
        for qt in range(2):
            for h in range(HG):
                qsl = slice(qt * HN, (qt + 1) * HN)
                # single-buffered: freed right after the hu/dn reads below
                ho = psho.tile([DH + 1, HN], F32, tag="hops", name="hops",
                               bufs=1)

                def emit_ho(mt, ex):
                    mm512(ho, Vr[:, mt, h, :], ex,
                          start=(mt == 0), stop=(mt == NMT - 1))

                # ho matmuls lag one mt behind exp so the single-buffered
                # ho slot wait (prev head's hu read) never blocks the sc/exp
                # stream at head boundaries
                pend_ho = None
                for mt in range(NMT):
                    sc = pssc.tile([128, HN], F32, tag="scps", name="scps")
                    mm512(sc, KTr[:, h, mt * 128:(mt + 1) * 128],
                          QTr[:, h, qsl], start=True, stop=True)
                    ex = pex.tile([128, HN], BF16, tag="ex", name="ex")
                    nc.scalar.activation(out=ex, in_=sc, func=AF.Exp, scale=SCALE)
                    if pend_ho is not None:
                        emit_ho(*pend_ho)
                    pend_ho = (mt, ex)
                    if qt == 1 and h == 0 and mt == 2 and pending is not None:
                        # emit the lagged (qt0, h7) normalize early: its
                        # reciprocal is long done, and it unblocks qt0
                        # outproj drains already during this head
                        normalize(*pending)
                        pending = None
                    # drain finished-half outproj tiles into the PE slack
                    # left by the ACT-bound exp stream, spread as 2-matmul
                    # chunks so no single insertion outruns the sc ring
                    if qt == 1 and (h >= 1 or mt >= 5) and mt in DRAIN_MT:
                        drain_chunk(DRAIN_MT[mt])
                emit_ho(*pend_ho)
                # free ho with ONE f32 read (numerator rows + denom row);
                # recip input goes through a partition-0 copy (the custom
                # DVE recip mishandles nonzero base partitions)
                hu = phur.tile([DH + 1, HN], F32, tag="hur", name="hur")
                nc.vector.tensor_copy(out=hu, in_=ho)
                dn = prcd.tile([1, HN], F32, tag="dn", name="dn")
                nc.vector.tensor_copy(out=dn, in_=hu[DH:DH + 1, :])
                rcf = prc.tile([1, HN], F32, tag="rcf", name="rcf")
                nc.vector.reciprocal_approx_fast(out=rcf, in_=dn)
                if qt == 1 and h == HG - 1:
                    # eager f32r conversion so the tail's broadcast matmul
                    # isn't gated on one more serial DVE op
                    final_rc = prc.tile([1, HN], F32R, tag="rcf2",
                                        name="rcf2", bufs=1)
                    nc.vector.tensor_copy(out=final_rc, in_=rcf)
                if pending is not None:
                    normalize(*pending)
                pending = (h, qt, rcf, hu)
            if qt == 0:
                outq = [(et, tt) for et in range(DIM // 512)
                        for tt in range(NMT // 2)]
        # tail: pre-start four tiles on contraction rows 0..4 (heads 0..6,
        # already normalized) so the PE stays busy through the final
        # normalize's reciprocal chain; then finish them and run the rest
        tail = [(et, tt) for tt in range(NMT // 2, NMT)
                for et in range(DIM // 512)]
        NPRE = 6
        pre_aps = []
        for _ in range(3):
            # each idle scps slab provides two independent half-bank groups
            slab = pssc.tile([128, HN], F32, tag="scps", name="pre_slab")
            pre_aps.append(slab[:, 0:512])
            pre_aps.append(slab[:, 512:1024])
        pre = []
        for (et, tt), ps in zip(tail[:NPRE], pre_aps):
            for c in range(RT - 1):
                nc.tensor.matmul(
                    out=ps, lhsT=hoP[:, c, tt * 128:(tt + 1) * 128],
                    rhs=wo[et][:, c, :], start=(c == 0), stop=False,
                )
            pre.append(ps)
        normalize(*pending, hops_bc=True)
        for (et, tt), ps in zip(tail[:NPRE], pre):
            nc.tensor.matmul(
                out=ps, lhsT=hoP[:, RT - 1, tt * 128:(tt + 1) * 128],
                rhs=wo[et][:, RT - 1, :], start=False, stop=True,
            )
            osb = posb.tile([128, 512], F32, tag="osb", name="osb")
            nc.vector.tensor_add(
                out=osb, in0=ps, in1=bias_sb[:, et * 512:(et + 1) * 512]
            )
            out_qs[(et + tt) % 2].dma_start(
                out=out[tt * 128:(tt + 1) * 128, et * 512:(et + 1) * 512],
                in_=osb,
            )
        for et, tt in outq:
            emit_outproj(et, tt)
        for et, tt in tail[NPRE:]:
            emit_outproj(et, tt)


def build():
    from contextlib import ExitStack

    nc = bacc.Bacc("TRN2", target_bir_lowering=False, debug=False)
    xbP = nc.declare_dram_parameter("xbP", [2 * 128, KT * HN], BF16, isOutput=False)
    cosP = nc.declare_dram_parameter("cosR", [128, 3, N], BF16, isOutput=False)
    sinP = nc.declare_dram_parameter("sinR", [128, 3, N], BF16, isOutput=False)
    ropePT = nc.declare_dram_parameter("ropePT", [128, 3 * 128], BF16, isOutput=False)
    WgT = nc.declare_dram_parameter("WgT", [128, 27648], BF16, isOutput=False)
    WoT = nc.declare_dram_parameter("WoT", [128, 3, RT * 512], BF16, isOutput=False)
    bout_p = nc.declare_dram_parameter("bout", [DIM], F32, isOutput=False)
    out = nc.declare_dram_parameter("out", [N, DIM], F32, isOutput=True)
    io = tuple(t[:] for t in (xbP, cosP, sinP, ropePT, WgT, WoT, bout_p, out))
    with ExitStack() as ctx:
        tc = ctx.enter_context(tile.TileContext(nc))
        _emit(ctx, tc, io)
    nc.finalize()
    return nc


def make_in_maps(x, f1, f2, f3, Wqkv, Wout, bout):
    x = np.asarray(x, np.float32)
    fcat = np.concatenate(
        [np.asarray(f1, np.float32), np.asarray(f2, np.float32),
         np.asarray(f3, np.float32)], axis=1,
    )  # [N, DH]
    idx = (128 * np.arange(3)[:, None] + np.arange(128)[None, :]) % 96  # [3, 128]
    fR = fcat.T[idx].transpose(1, 0, 2)  # [128, 3, N]
    cosR_np = np.ascontiguousarray(np.cos(fR)).astype(IN_NP)
    sinR_np = np.ascontiguousarray(np.sin(fR)).astype(IN_NP)
    PT = _build_rope_pt_packed().astype(IN_NP)
    Wqkv = np.asarray(Wqkv, np.float32)
    Wout = np.asarray(Wout, np.float32)
    bout = np.ascontiguousarray(np.asarray(bout, np.float32))
    zeros_bias = np.zeros_like(bout)
    # x pre-tiled: [2, 128, KT, HN] -> [256, KT*HN]; per-partition contiguous
    xP = []
    for b in range(B):
        xb = x[b].T.reshape(KT, 128, 2, HN)           # [k, p, half, n]
        xb = xb.transpose(2, 1, 0, 3).reshape(2 * 128, KT * HN)
        xP.append(np.ascontiguousarray(xb).astype(IN_NP))
    WgT_g, WoT_g = [], []
    for g in range(2):
        cols = np.concatenate(
            [Wqkv[:, g * G:(g + 1) * G],
             Wqkv[:, H * DH + g * G:H * DH + (g + 1) * G],
             Wqkv[:, 2 * H * DH + g * G:2 * H * DH + (g + 1) * G]], axis=1,
        )  # [DIM, 3G]: [Q | K | V] for this group
        W3 = cols.reshape(KT, 128, 3 * G)
        blocks = []
        for ct in range(2):  # V tiles
            cs = 2 * G + ct * 384
            blocks.append(W3[:, :, cs:cs + 384].transpose(1, 0, 2).reshape(128, KT * 384))
        for si, base in ((0, G), (1, 0)):  # K tiles then Q tiles
            for rt in range(RT):
                cs = base + rt * 128
                blocks.append(W3[:, :, cs:cs + 128].transpose(1, 0, 2).reshape(128, KT * 128))
        WgT_g.append(np.ascontiguousarray(np.concatenate(blocks, axis=1)).astype(IN_NP))
        WoG_ = Wout[g * G:(g + 1) * G, :]
        Wo3 = WoG_.reshape(RT, 128, DIM)
        ets = [Wo3[:, :, et * 512:(et + 1) * 512].transpose(1, 0, 2).reshape(128, 1, RT * 512)
               for et in range(3)]
        WoT_g.append(np.ascontiguousarray(np.concatenate(ets, axis=1)).astype(IN_NP))
    in_maps = []
    for c in range(8):
        b, g = divmod(c, 2)
        in_maps.append(dict(
            xbP=xP[b], cosR=cosR_np, sinR=sinR_np, ropePT=PT,
            WgT=WgT_g[g], WoT=WoT_g[g],
            bout=(bout if g == 0 else zeros_bias),
        ))
    return in_maps


_NC_CACHE = None


def kernel(x, f1, f2, f3, Wqkv, Wout, bout, _trace=False):
    global _NC_CACHE
    if _NC_CACHE is None:
        _NC_CACHE = build()
    nc = _NC_CACHE
    in_maps = make_in_maps(x, f1, f2, f3, Wqkv, Wout, bout)
    res = run_bass_kernel_spmd(nc, in_maps, list(range(8)), trace=_trace)
    out = np.empty((B, N, DIM), np.float32)
    for b in range(B):
        out[b] = res.results[2 * b]["out"]
        out[b] += res.results[2 * b + 1]["out"]
    if _trace:
        return out, res
    return out



# revision 70
# speedup vs baseline: 1.0060x; 1.0060x over previous
"""Trainium2 Bass kernel: 3D-RoPE multi-head attention (B=4,N=2048,DIM=1536,H=16,DH=96).

Sharding: 8 cores = (batch b = c//2) x (head-group g = c%2, 8 heads each).
Each core:
  - projects Q,K,V for its 8 heads over all 2048 tokens (tensor parallel:
    Wqkv column-split, Wout row-split). K/Q projected in PACKED 128-row
    tiles of the [768, N] K^T/Q^T matrix (full PE-array utilization), RoPE
    applied per packed tile (3 distinct 128x128 sign-permutations,
    host-precomputed replicated cos/sin), then DMA-unpacked into per-head
    [96, N] tiles.
  - attention per head (softmax over keys, no max-subtraction), query-half
    outer loop; output-projection tiles for the finished query half are
    interleaved into the next half's head loop to fill the PE slack left
    by the ACT-bound exp stream (outproj PSUM shares the ho pool ring).
  - normalized head outputs DMA-packed into [128, 6, N] tiles; partial
    output projection contracts 6x128 rows of Wout.
Host gather: out[b] = core(2b) + core(2b+1) partial sums (bias on g=0).
All matmul inputs bf16, fp32 PSUM accumulation, no DRAM spills.
"""

import sys

if "/opt/trn_rl_repo" not in sys.path:
    sys.path.insert(0, "/opt/trn_rl_repo")

import numpy as np

import concourse.bass as bass
import concourse.mybir as mybir
import concourse.tile as tile
from concourse import bacc
from concourse.bass_utils import run_bass_kernel_spmd

B, N, DIM, H, DH = 4, 2048, 1536, 16, 96
HG = 8            # heads per core
G = HG * DH       # 768 rows/cols per group
RT = G // 128     # 6 packed row tiles
KT = DIM // 128   # 12 contraction tiles
HN = N // 2       # token half (projection passes) / query chunk
NMT = N // 128    # 16 key tiles
SCALE = DH ** -0.5
F32 = mybir.dt.float32
F32R = mybir.dt.float32r
BF16 = mybir.dt.bfloat16
F16 = mybir.dt.float16
import ml_dtypes
IN_NP = ml_dtypes.bfloat16
AF = mybir.ActivationFunctionType

# packed-tile segment maps (128-row tile j of a [384,*] block, period 3)
# replica rows: row r of tile j = master row (128*j + r) % 96
REP_SEGS = {
    0: [(0, 96, 0), (96, 128, 0)],
    1: [(0, 64, 32), (64, 128, 0)],
    2: [(0, 32, 64), (32, 128, 0)],
}
# per packed tile rt: (head, tile_lo, tile_hi, head_row_lo)
def _tile_segs(rt):
    segs = []
    r = 128 * rt
    while r < 128 * rt + 128:
        h = r // 96
        end = min(128 * rt + 128, (h + 1) * 96)
        segs.append((h, r - 128 * rt, end - 128 * rt, r % 96))
        r = end
    return segs
TILE_SEGS = {rt: _tile_segs(rt) for rt in range(RT)}
# per head: list of (rt, tile_lo, tile_hi, head_row_lo)
HEAD_SEGS = {h: [] for h in range(HG)}
for rt in range(RT):
    for h, a, b, d0 in TILE_SEGS[rt]:
        HEAD_SEGS[h].append((rt, a, b, d0))


def _build_rope_pt_packed() -> np.ndarray:
    """lhsT tiles for rotate_half on packed 128-row layout: A[:, j*128:(j+1)*128]
    = P_j^T where rot_packed = P_j @ t_packed for tile j (j = rt % 3)."""
    A = np.zeros((128, 3 * 128), np.float32)
    for j in range(3):
        for r in range(128):
            Rg = 128 * j + r
            d = Rg % 96
            c, a = d // 32, d % 32
            sign = -1.0 if a < 16 else 1.0
            dq = 32 * c + (a + 16) % 32
            qg = (Rg // 96) * 96 + dq
            ql = qg - 128 * j
            # P_j[r, ql] = sign; lhsT[k, i] = P_j[i, k]
            A[ql, j * 128 + r] = sign
    return np.ascontiguousarray(A)


def _emit(ctx, tc, io):
    nc = tc.nc
    xbP, cosP, sinP, ropePT, WgT, WoT, bout, out = io

    def mm512(out_ap, lhsT, rhs, start, stop, width=HN):
        for c0 in range(0, width, 512):
            nc.tensor.matmul(
                out=out_ap[:, c0:c0 + 512], lhsT=lhsT,
                rhs=rhs[:, c0:c0 + 512], start=start, stop=stop,
            )

    persist = ctx.enter_context(tc.tile_pool(name="persist", bufs=1))

    # ---- constants ------------------------------------------------------
    ropeP_sb = persist.tile([128, 3, 128], BF16, tag="ropeP")
    nc.sync.dma_start(out=ropeP_sb, in_=ropePT.rearrange("p (j c) -> p j c", j=3))
    ones1f = persist.tile([1, DH], F32, tag="ones1f")
    nc.vector.memset(ones1f, 1.0)
    ones1 = persist.tile([1, DH], F32R, tag="ones1")
    nc.scalar.copy(out=ones1, in_=ones1f)

    # resident tensors
    KTr = persist.tile([DH, HG, N], BF16, tag="KTr")
    QTr = persist.tile([DH, HG, N], BF16, tag="QTr")
    Vr = persist.tile([128, NMT, HG, DH + 1], BF16, tag="Vr")
    nc.vector.memset(Vr[:, :, :, DH:DH + 1], 1.0)

    # ---- projections (two token-half passes, shared xb tiles) ------------
    # WgT columns: [V ct0 | V ct1 | K rt0..5 | Q rt0..5], each tile k-major
    # and per-partition contiguous. xbP rows: [p*128 + partition].
    with tc.tile_pool(name="wv", bufs=1) as pwv:
        # V weights are token-independent: both column tiles load once for
        # both passes; wv0 leads the scalar queue (first-matmul critical
        # path), wv1 follows p0's xb1
        wvt = [pwv.tile([128, KT, 384], BF16, tag=f"wv{ct}", name=f"wv{ct}")
               for ct in range(2)]
        nc.scalar.dma_start(out=wvt[0], in_=WgT[:, 0:4608])
        # host-precomputed replicated cos/sin for packed rope [128, 3, N];
        # projection-scoped so attention inherits the 24KB
        cosR = pwv.tile([128, 3, N], BF16, tag="cosR")
        sinR = pwv.tile([128, 3, N], BF16, tag="sinR")

        rope_pending = None

        def emit_rope(ps, dst, tok):
            """Packed rope on finished projection group ps -> dst per-head."""
            rt = ps_rt[id(ps)]
            j = rt % 3
            t_sb = prp.tile([128, HN], BF16, tag="rope_t", name="rope_t")
            nc.vector.tensor_copy(out=t_sb, in_=ps)
            rot = pskr.tile([128, HN], F32, tag="rope_rot", name="rope_rot")
            mm512(rot, ropeP_sb[:, j, :], t_sb, start=True, stop=True)
            u = prp.tile([128, HN], BF16, tag="rope_u", name="rope_u")
            nc.vector.tensor_mul(out=u, in0=t_sb, in1=cosR[:, j, tok])
            nc.vector.tensor_mul(out=rot, in0=rot, in1=sinR[:, j, tok])
            st = prp.tile([128, HN], BF16, tag="rope_st", name="rope_st")
            nc.vector.tensor_add(out=st, in0=u, in1=rot)
            for (h, a, b, d0) in TILE_SEGS[rt]:
                nc.sync.dma_start(
                    out=dst[d0:d0 + (b - a), h, tok], in_=st[a:b, :]
                )

        ps_rt = {}
        with (
            tc.tile_pool(name="xb", bufs=2) as pxb,
            tc.tile_pool(name="wkq", bufs=3) as pwkq,
            tc.tile_pool(name="vst", bufs=6) as pvst,
            tc.tile_pool(name="rope", bufs=2) as prp,
            tc.tile_pool(name="psk", bufs=2, space="PSUM") as psk,
            tc.tile_pool(name="pskr", bufs=1, space="PSUM") as pskr,
        ):
            for p in range(2):
                tok = slice(p * HN, (p + 1) * HN)
                xb3 = []
                xb_q = (nc.sync, nc.scalar, nc.sync)
                for i in range(3):
                    t3 = pxb.tile([128, 4, HN], BF16, tag=f"xb{i}",
                                  name=f"xb{p}_{i}")
                    xb_q[i].dma_start(
                        out=t3,
                        in_=xbP[p * 128:(p + 1) * 128, i * 4 * HN:(i + 1) * 4 * HN],
                    )
                    xb3.append(t3)
                if p == 0:
                    # wv1 after p0's xb1 on scalar (queue transfers
                    # serialize; first ct1 use is ~27us in)
                    nc.scalar.dma_start(out=wvt[1], in_=WgT[:, 4608:9216])
                xb = [xb3[k // 4][:, k % 4, :] for k in range(KT)]
                # V projection: [token, vcol] tiles, 2 col tiles of 384.
                # PSUM accumulation is order-independent: consume xb tiles
                # in DMA-arrival order (xb0 sync, xb2 sync, xb1 scalar) so
                # the first chains start before all of x has landed
                KORD = [0, 1, 2, 3, 8, 9, 10, 11, 4, 5, 6, 7]
                for ct in range(2):
                    wv = wvt[ct]
                    for tt in range(HN // 128):
                        ps = psk.tile([128, 384], F32, tag="kps", name="vps")
                        for ki, k in enumerate(KORD):
                            nc.tensor.matmul(
                                out=ps, lhsT=xb[k][:, tt * 128:(tt + 1) * 128],
                                rhs=wv[:, k, :], start=(ki == 0),
                                stop=(ki == KT - 1),
                            )
                        vst = pvst.tile([128, 4, DH], BF16, tag="vst", name="vst")
                        nc.vector.tensor_copy(out=vst, in_=ps)
                        mt = p * (HN // 128) + tt
                        # alternate staging queues to halve queue latency
                        (nc.sync if tt % 2 == 0 else nc.scalar).dma_start(
                            out=Vr[:, mt, ct * 4:(ct + 1) * 4, 0:DH], in_=vst
                        )
                # K and Q projection, packed 128-row tiles + lagged rope
                for dst, si, nm in ((KTr, 0, "k"), (QTr, 1, "q")):
                    for rt in range(RT):
                        cs = 9216 + si * 9216 + rt * 1536
                        wk = pwkq.tile([128, KT, 128], BF16, tag="wkq",
                                       name=f"w{nm}{p}_{rt}")
                        nc.scalar.dma_start(out=wk, in_=WgT[:, cs:cs + 1536])
                        if p == 0 and si == 0 and rt == 0:
                            # trig behind the first K weights: its transfer
                            # must not race the startup-critical loads
                            # (DMA bandwidth is shared across queues);
                            # first rope use is ~45us in
                            nc.scalar.dma_start(out=cosR, in_=cosP)
                            nc.scalar.dma_start(out=sinR, in_=sinP)
                        ps = psk.tile([128, HN], F32, tag="kps", name="kps")
                        ps_rt[id(ps)] = rt
                        for k in range(KT):
                            mm512(ps, wk[:, k, :], xb[k], start=(k == 0),
                                  stop=(k == KT - 1))
                        if rope_pending is not None:
                            emit_rope(*rope_pending)
                        rope_pending = (ps, dst, tok)
            emit_rope(*rope_pending)

    # ---- attention + packed ho + interleaved output projection -----------
    span = ctx.enter_context(tc.tile_pool(name="span", bufs=1))
    hoP = span.tile([128, RT, N], BF16, tag="hoP")
    bias_sb = span.tile([128, DIM], F32, tag="bias")
    bout_bc = bass.AP(tensor=bout.tensor, offset=bout.offset,
                      ap=[[0, 128]] + [list(p) for p in bout.ap])
    nc.sync.dma_start(out=bias_sb, in_=bout_bc)
    wo = []
    for et in range(DIM // 512):
        wt = span.tile([128, RT, 512], BF16, tag=f"wo{et}", name=f"wo{et}")
        nc.sync.dma_start(out=wt, in_=WoT[:, et, :])
        wo.append(wt)
    with (
        tc.tile_pool(name="ex", bufs=4) as pex,
        tc.tile_pool(name="hur", bufs=3) as phur,
        tc.tile_pool(name="stn", bufs=2) as pstn,
        tc.tile_pool(name="rcd", bufs=1) as prcd,
        tc.tile_pool(name="rc", bufs=2) as prc,
        tc.tile_pool(name="osb", bufs=4) as posb,
        tc.tile_pool(name="rbd", bufs=2, space="DRAM") as prbd,
        tc.tile_pool(name="bcs", bufs=2) as pbcs,
        tc.tile_pool(name="pssc", bufs=3, space="PSUM") as pssc,
        tc.tile_pool(name="psho", bufs=2, space="PSUM") as psho,
    ):
        def normalize(h, qt, rcf, hu, hops_bc=False):
            """stage = hu * broadcast(rc); DMA-pack staged rows into hoP.

            The broadcast runs in the eps ring (idle during attention) so it
            never shrinks the sc ring's pipeline depth; the final flush uses
            the freed ho slot instead (eps+scps hold open pre-tail groups).
            """
            qsl = slice(qt * HN, (qt + 1) * HN)
            stn = pstn.tile([DH, HN], BF16, tag="stn", name="stn")
            if hops_bc:
                # final flush: PE is idle then, and the matmul broadcast is
                # lower-latency than the DRAM bounce; final_rc was converted
                # to f32r eagerly inside the last head's chain
                bc = psho.tile([DH, HN], F32, tag="hops", name="bcps",
                               bufs=1)
                mm512(bc, ones1, final_rc, start=True, stop=True)
                nc.vector.tensor_mul(out=stn, in0=hu[0:DH, :], in1=bc)
            else:
                # steady state: broadcast via DRAM bounce (stride-0 reads
                # are legal from DRAM) — zero PE work, latency hidden by
                # the one-head normalize lag
                rb = prbd.tile([1, HN], F32, tag="rbd", name="rbd")
                nc.sync.dma_start(out=rb, in_=rcf)
                rb_bc = bass.AP(tensor=rb.tensor, offset=rb.offset,
                                ap=[[0, DH]] + [list(p) for p in rb.ap[1:]])
                bc = pbcs.tile([DH, HN], F32, tag="bcs", name="bcs")
                nc.sync.dma_start(out=bc, in_=rb_bc)
                nc.vector.tensor_mul(out=stn, in0=hu[0:DH, :], in1=bc)
            for (rt, a, b, d0) in HEAD_SEGS[h]:
                nc.sync.dma_start(
                    out=hoP[a:b, rt, qsl], in_=stn[d0:d0 + (b - a), :]
                )

        out_qs = [nc.sync, nc.scalar]

        def emit_outproj(et, tt):
            """One [128-token, 512-col] output tile; own PSUM ring."""
            ps = pssc.tile([128, 512], F32, tag="scps", name="eps")
            for c in range(RT):
                nc.tensor.matmul(
                    out=ps, lhsT=hoP[:, c, tt * 128:(tt + 1) * 128],
                    rhs=wo[et][:, c, :], start=(c == 0), stop=(c == RT - 1),
                )
            osb = posb.tile([128, 512], F32, tag="osb", name="osb")
            nc.vector.tensor_add(
                out=osb, in0=ps, in1=bias_sb[:, et * 512:(et + 1) * 512]
            )
            out_qs[(et + tt) % 2].dma_start(
                out=out[tt * 128:(tt + 1) * 128, et * 512:(et + 1) * 512],
                in_=osb,
            )

        outq = []
        # mt slot -> chunk phase for spread-out drains
        DRAIN_MT = {5: 0, 6: 1, 7: 2, 10: 0, 11: 1, 12: 2, 13: 0, 14: 1,
                    15: 2}
        cur_drain = [None]

        def drain_chunk(phase):
            if phase == 0:
                if not outq:
                    cur_drain[0] = None
                    return
                et, tt = outq.pop(0)
                ps = pssc.tile([128, 512], F32, tag="scps", name="eps")
                cur_drain[0] = (et, tt, ps)
            if cur_drain[0] is None:
                return
            et, tt, ps = cur_drain[0]
            for c in (2 * phase, 2 * phase + 1):
                nc.tensor.matmul(
                    out=ps, lhsT=hoP[:, c, tt * 128:(tt + 1) * 128],
                    rhs=wo[et][:, c, :], start=(c == 0), stop=(c == RT - 1),
                )
            if phase == 2:
                osb = posb.tile([128, 512], F32, tag="osb", name="osb")
                nc.vector.tensor_add(
                    out=osb, in0=ps, in1=bias_sb[:, et * 512:(et + 1) * 512]
                )
                out_qs[(et + tt) % 2].dma_start(
                    out=out[tt * 128:(tt + 1) * 128,
                            et * 512:(et + 1) * 512],
                    in_=osb,
                )
                cur_drain[0] = None

        pending = None
        for qt in range(2):
            for h in range(HG):
                qsl = slice(qt * HN, (qt + 1) * HN)
                # single-buffered: freed right after the hu/dn reads below
                ho = psho.tile([DH + 1, HN], F32, tag="hops", name="hops",
                               bufs=1)

                def emit_ho(mt, ex):
                    mm512(ho, Vr[:, mt, h, :], ex,
                          start=(mt == 0), stop=(mt == NMT - 1))

                # ho matmuls lag one mt behind exp so the single-buffered
                # ho slot wait (prev head's hu read) never blocks the sc/exp
                # stream at head boundaries
                pend_ho = None
                for mt in range(NMT):
                    sc = pssc.tile([128, HN], F32, tag="scps", name="scps")
                    mm512(sc, KTr[:, h, mt * 128:(mt + 1) * 128],
                          QTr[:, h, qsl], start=True, stop=True)
                    ex = pex.tile([128, HN], BF16, tag="ex", name="ex")
                    nc.scalar.activation(out=ex, in_=sc, func=AF.Exp, scale=SCALE)
                    if pend_ho is not None:
                        emit_ho(*pend_ho)
                    pend_ho = (mt, ex)
                    if qt == 1 and h == 0 and mt == 2 and pending is not None:
                        # emit the lagged (qt0, h7) normalize early: its
                        # reciprocal is long done, and it unblocks qt0
                        # outproj drains already during this head
                        normalize(*pending)
                        pending = None
                    # drain finished-half outproj tiles into the PE slack
                    # left by the ACT-bound exp stream, spread as 2-matmul
                    # chunks so no single insertion outruns the sc ring
                    if qt == 1 and (h >= 1 or mt >= 5) and mt in DRAIN_MT:
                        drain_chunk(DRAIN_MT[mt])
                emit_ho(*pend_ho)
                # free ho with ONE f32 read (numerator rows + denom row);
                # recip input goes through a partition-0 copy (the custom
                # DVE recip mishandles nonzero base partitions)
                hu = phur.tile([DH + 1, HN], F32, tag="hur", name="hur")
                nc.vector.tensor_copy(out=hu, in_=ho)
                dn = prcd.tile([1, HN], F32, tag="dn", name="dn")
                nc.vector.tensor_copy(out=dn, in_=hu[DH:DH + 1, :])
                rcf = prc.tile([1, HN], F32, tag="rcf", name="rcf")
                nc.vector.reciprocal_approx_fast(out=rcf, in_=dn)
                if qt == 1 and h == HG - 1:
                    # eager f32r conversion so the tail's broadcast matmul
                    # isn't gated on one more serial DVE op
                    final_rc = prc.tile([1, HN], F32R, tag="rcf2",
                                        name="rcf2", bufs=1)
                    nc.vector.tensor_copy(out=final_rc, in_=rcf)
                if pending is not None:
                    normalize(*pending)
                pending = (h, qt, rcf, hu)
            if qt == 0:
                outq = [(et, tt) for et in range(DIM // 512)
                        for tt in range(NMT // 2)]
        # tail: pre-start four tiles on contraction rows 0..4 (heads 0..6,
        # already normalized) so the PE stays busy through the final
        # normalize's reciprocal chain; then finish them and run the rest
        tail = [(et, tt) for tt in range(NMT // 2, NMT)
                for et in range(DIM // 512)]
        NPRE = 6
        pre_aps = []
        for _ in range(3):
            # each idle scps slab provides two independent half-bank groups
            slab = pssc.tile([128, HN], F32, tag="scps", name="pre_slab")
            pre_aps.append(slab[:, 0:512])
            pre_aps.append(slab[:, 512:1024])
        pre = []
        for (et, tt), ps in zip(tail[:NPRE], pre_aps):
            for c in range(RT - 1):
                nc.tensor.matmul(
                    out=ps, lhsT=hoP[:, c, tt * 128:(tt + 1) * 128],
                    rhs=wo[et][:, c, :], start=(c == 0), stop=False,
                )
            pre.append(ps)
        normalize(*pending, hops_bc=True)
        for (et, tt), ps in zip(tail[:NPRE], pre):
            nc.tensor.matmul(
                out=ps, lhsT=hoP[:, RT - 1, tt * 128:(tt + 1) * 128],
                rhs=wo[et][:, RT - 1, :], start=False, stop=True,
            )
            osb = posb.tile([128, 512], F32, tag="osb", name="osb")
            nc.vector.tensor_add(
                out=osb, in0=ps, in1=bias_sb[:, et * 512:(et + 1) * 512]
            )
            out_qs[(et + tt) % 2].dma_start(
                out=out[tt * 128:(tt + 1) * 128, et * 512:(et + 1) * 512],
                in_=osb,
            )
        for et, tt in outq:
            emit_outproj(et, tt)
        for et, tt in tail[NPRE:]:
            emit_outproj(et, tt)


def build():
    from contextlib import ExitStack

    nc = bacc.Bacc("TRN2", target_bir_lowering=False, debug=False)
    xbP = nc.declare_dram_parameter("xbP", [2 * 128, KT * HN], BF16, isOutput=False)
    cosP = nc.declare_dram_parameter("cosR", [128, 3, N], BF16, isOutput=False)
    sinP = nc.declare_dram_parameter("sinR", [128, 3, N], BF16, isOutput=False)
    ropePT = nc.declare_dram_parameter("ropePT", [128, 3 * 128], BF16, isOutput=False)
    WgT = nc.declare_dram_parameter("WgT", [128, 27648], BF16, isOutput=False)
    WoT = nc.declare_dram_parameter("WoT", [128, 3, RT * 512], BF16, isOutput=False)
    bout_p = nc.declare_dram_parameter("bout", [DIM], F32, isOutput=False)
    out = nc.declare_dram_parameter("out", [N, DIM], F32, isOutput=True)
    io = tuple(t[:] for t in (xbP, cosP, sinP, ropePT, WgT, WoT, bout_p, out))
    with ExitStack() as ctx:
        tc = ctx.enter_context(tile.TileContext(nc))
        _emit(ctx, tc, io)
    nc.finalize()
    return nc


def make_in_maps(x, f1, f2, f3, Wqkv, Wout, bout):
    x = np.asarray(x, np.float32)
    fcat = np.concatenate(
        [np.asarray(f1, np.float32), np.asarray(f2, np.float32),
         np.asarray(f3, np.float32)], axis=1,
    )  # [N, DH]
    idx = (128 * np.arange(3)[:, None] + np.arange(128)[None, :]) % 96  # [3, 128]
    fR = fcat.T[idx].transpose(1, 0, 2)  # [128, 3, N]
    cosR_np = np.ascontiguousarray(np.cos(fR)).astype(IN_NP)
    sinR_np = np.ascontiguousarray(np.sin(fR)).astype(IN_NP)
    PT = _build_rope_pt_packed().astype(IN_NP)
    Wqkv = np.asarray(Wqkv, np.float32)
    Wout = np.asarray(Wout, np.float32)
    bout = np.ascontiguousarray(np.asarray(bout, np.float32))
    zeros_bias = np.zeros_like(bout)
    # x pre-tiled: [2, 128, KT, HN] -> [256, KT*HN]; per-partition contiguous
    xP = []
    for b in range(B):
        xb = x[b].T.reshape(KT, 128, 2, HN)           # [k, p, half, n]
        xb = xb.transpose(2, 1, 0, 3).reshape(2 * 128, KT * HN)
        xP.append(np.ascontiguousarray(xb).astype(IN_NP))
    WgT_g, WoT_g = [], []
    for g in range(2):
        cols = np.concatenate(
            [Wqkv[:, g * G:(g + 1) * G],
             Wqkv[:, H * DH + g * G:H * DH + (g + 1) * G],
             Wqkv[:, 2 * H * DH + g * G:2 * H * DH + (g + 1) * G]], axis=1,
        )  # [DIM, 3G]: [Q | K | V] for this group
        W3 = cols.reshape(KT, 128, 3 * G)
        blocks = []
        for ct in range(2):  # V tiles
            cs = 2 * G + ct * 384
            blocks.append(W3[:, :, cs:cs + 384].transpose(1, 0, 2).reshape(128, KT * 384))
        for si, base in ((0, G), (1, 0)):  # K tiles then Q tiles
            for rt in range(RT):
                cs = base + rt * 128
                blocks.append(W3[:, :, cs:cs + 128].transpose(1, 0, 2).reshape(128, KT * 128))
        WgT_g.append(np.ascontiguousarray(np.concatenate(blocks, axis=1)).astype(IN_NP))
        WoG_ = Wout[g * G:(g + 1) * G, :]
        Wo3 = WoG_.reshape(RT, 128, DIM)
        ets = [Wo3[:, :, et * 512:(et + 1) * 512].transpose(1, 0, 2).reshape(128, 1, RT * 512)
               for et in range(3)]
        WoT_g.append(np.ascontiguousarray(np.concatenate(ets, axis=1)).astype(IN_NP))
    in_maps = []
    for c in range(8):
        b, g = divmod(c, 2)
        in_maps.append(dict(
            xbP=xP[b], cosR=cosR_np, sinR=sinR_np, ropePT=PT,
            WgT=WgT_g[g], WoT=WoT_g[g],
            bout=(bout if g == 0 else zeros_bias),
        ))
    return in_maps


_NC_CACHE = None


def kernel(x, f1, f2, f3, Wqkv, Wout, bout, _trace=False):
    global _NC_CACHE
    if _NC_CACHE is None:
        _NC_CACHE = build()
    nc = _NC_CACHE
    in_maps = make_in_maps(x, f1, f2, f3, Wqkv, Wout, bout)
    res = run_bass_kernel_spmd(nc, in_maps, list(range(8)), trace=_trace)
    out = np.empty((B, N, DIM), np.float32)
    for b in range(B):
        out[b] = res.results[2 * b]["out"]
        out[b] += res.results[2 * b + 1]["out"]
    if _trace:
        return out, res
    return out



# revision 77
# speedup vs baseline: 1.0078x; 1.0018x over previous
"""Trainium2 Bass kernel: 3D-RoPE multi-head attention (B=4,N=2048,DIM=1536,H=16,DH=96).

Sharding: 8 cores = (batch b = c//2) x (head-group g = c%2, 8 heads each).
Each core:
  - projects Q,K,V for its 8 heads over all 2048 tokens (tensor parallel:
    Wqkv column-split, Wout row-split). K/Q projected in PACKED 128-row
    tiles of the [768, N] K^T/Q^T matrix (full PE-array utilization), RoPE
    applied per packed tile (3 distinct 128x128 sign-permutations,
    host-precomputed replicated cos/sin), then DMA-unpacked into per-head
    [96, N] tiles.
  - attention per head (softmax over keys, no max-subtraction), query-half
    outer loop; output-projection tiles for the finished query half are
    interleaved into the next half's head loop to fill the PE slack left
    by the ACT-bound exp stream (outproj PSUM shares the ho pool ring).
  - normalized head outputs DMA-packed into [128, 6, N] tiles; partial
    output projection contracts 6x128 rows of Wout.
Host gather: out[b] = core(2b) + core(2b+1) partial sums (bias on g=0).
All matmul inputs bf16, fp32 PSUM accumulation, no DRAM spills.
"""

import sys

if "/opt/trn_rl_repo" not in sys.path:
    sys.path.insert(0, "/opt/trn_rl_repo")

import numpy as np

import concourse.bass as bass
import concourse.mybir as mybir
import concourse.tile as tile
from concourse import bacc
from concourse.bass_utils import run_bass_kernel_spmd

B, N, DIM, H, DH = 4, 2048, 1536, 16, 96
HG = 8            # heads per core
G = HG * DH       # 768 rows/cols per group
RT = G // 128     # 6 packed row tiles
KT = DIM // 128   # 12 contraction tiles
HN = N // 2       # token half (projection passes) / query chunk
NMT = N // 128    # 16 key tiles
SCALE = DH ** -0.5
F32 = mybir.dt.float32
F32R = mybir.dt.float32r
BF16 = mybir.dt.bfloat16
F16 = mybir.dt.float16
import ml_dtypes
IN_NP = ml_dtypes.bfloat16
AF = mybir.ActivationFunctionType

# packed-tile segment maps (128-row tile j of a [384,*] block, period 3)
# replica rows: row r of tile j = master row (128*j + r) % 96
REP_SEGS = {
    0: [(0, 96, 0), (96, 128, 0)],
    1: [(0, 64, 32), (64, 128, 0)],
    2: [(0, 32, 64), (32, 128, 0)],
}
# per packed tile rt: (head, tile_lo, tile_hi, head_row_lo)
def _tile_segs(rt):
    segs = []
    r = 128 * rt
    while r < 128 * rt + 128:
        h = r // 96
        end = min(128 * rt + 128, (h + 1) * 96)
        segs.append((h, r - 128 * rt, end - 128 * rt, r % 96))
        r = end
    return segs
TILE_SEGS = {rt: _tile_segs(rt) for rt in range(RT)}
# per head: list of (rt, tile_lo, tile_hi, head_row_lo)
HEAD_SEGS = {h: [] for h in range(HG)}
for rt in range(RT):
    for h, a, b, d0 in TILE_SEGS[rt]:
        HEAD_SEGS[h].append((rt, a, b, d0))


def _build_rope_pt_packed() -> np.ndarray:
    """lhsT tiles for rotate_half on packed 128-row layout: A[:, j*128:(j+1)*128]
    = P_j^T where rot_packed = P_j @ t_packed for tile j (j = rt % 3)."""
    A = np.zeros((128, 3 * 128), np.float32)
    for j in range(3):
        for r in range(128):
            Rg = 128 * j + r
            d = Rg % 96
            c, a = d // 32, d % 32
            sign = -1.0 if a < 16 else 1.0
            dq = 32 * c + (a + 16) % 32
            qg = (Rg // 96) * 96 + dq
            ql = qg - 128 * j
            # P_j[r, ql] = sign; lhsT[k, i] = P_j[i, k]
            A[ql, j * 128 + r] = sign
    return np.ascontiguousarray(A)


def _emit(ctx, tc, io):
    nc = tc.nc
    xbP, cosP, sinP, ropePT, WgT, WoT, bout, out = io

    def mm512(out_ap, lhsT, rhs, start, stop, width=HN):
        for c0 in range(0, width, 512):
            nc.tensor.matmul(
                out=out_ap[:, c0:c0 + 512], lhsT=lhsT,
                rhs=rhs[:, c0:c0 + 512], start=start, stop=stop,
            )

    persist = ctx.enter_context(tc.tile_pool(name="persist", bufs=1))

    # ---- constants ------------------------------------------------------
    ropeP_sb = persist.tile([128, 3, 128], BF16, tag="ropeP")
    nc.sync.dma_start(out=ropeP_sb, in_=ropePT.rearrange("p (j c) -> p j c", j=3))
    ones1f = persist.tile([1, DH], F32, tag="ones1f")
    nc.vector.memset(ones1f, 1.0)
    ones1 = persist.tile([1, DH], F32R, tag="ones1")
    nc.scalar.copy(out=ones1, in_=ones1f)

    # resident tensors
    KTr = persist.tile([DH, HG, N], BF16, tag="KTr")
    QTr = persist.tile([DH, HG, N], BF16, tag="QTr")
    Vr = persist.tile([128, NMT, HG, DH + 1], BF16, tag="Vr")
    nc.vector.memset(Vr[:, :, :, DH:DH + 1], 1.0)

    # ---- projections (two token-half passes, shared xb tiles) ------------
    # WgT columns: [V ct0 | V ct1 | K rt0..5 | Q rt0..5], each tile k-major
    # and per-partition contiguous. xbP rows: [p*128 + partition].
    with tc.tile_pool(name="wv", bufs=1) as pwv:
        # V weights are token-independent: both column tiles load once for
        # both passes; wv0 leads the scalar queue (first-matmul critical
        # path), wv1 follows p0's xb1
        wvt = [pwv.tile([128, KT, 384], BF16, tag=f"wv{ct}", name=f"wv{ct}")
               for ct in range(2)]
        nc.scalar.dma_start(out=wvt[0], in_=WgT[:, 0:4608])
        # host-precomputed replicated cos/sin for packed rope [128, 3, N];
        # projection-scoped so attention inherits the 24KB
        cosR = pwv.tile([128, 3, N], BF16, tag="cosR")
        sinR = pwv.tile([128, 3, N], BF16, tag="sinR")

        rope_pending = None

        def capture_rope(ps):
            """Free the projection PSUM group immediately: the copy runs
            ahead of the lagged rope's muls on the in-order DVE queue, so
            bank release (and the proj->attention handoff) isn't serialized
            behind trig work."""
            t_sb = prp.tile([128, HN], BF16, tag="rope_t", name="rope_t")
            nc.vector.tensor_copy(out=t_sb, in_=ps)
            return t_sb

        def emit_rope(t_sb, rt, dst, tok):
            """Packed rope on a captured projection tile -> dst per-head."""
            j = rt % 3
            rot = pskr.tile([128, HN], F32, tag="rope_rot", name="rope_rot")
            mm512(rot, ropeP_sb[:, j, :], t_sb, start=True, stop=True)
            u = prp.tile([128, HN], BF16, tag="rope_u", name="rope_u")
            nc.vector.tensor_mul(out=u, in0=t_sb, in1=cosR[:, j, tok])
            nc.vector.tensor_mul(out=rot, in0=rot, in1=sinR[:, j, tok])
            st = prp.tile([128, HN], BF16, tag="rope_st", name="rope_st")
            nc.vector.tensor_add(out=st, in0=u, in1=rot)
            for (h, a, b, d0) in TILE_SEGS[rt]:
                nc.sync.dma_start(
                    out=dst[d0:d0 + (b - a), h, tok], in_=st[a:b, :]
                )

        with (
            tc.tile_pool(name="xb", bufs=2) as pxb,
            tc.tile_pool(name="wkq", bufs=3) as pwkq,
            tc.tile_pool(name="vst", bufs=8) as pvst,
            tc.tile_pool(name="rope", bufs=2) as prp,
            tc.tile_pool(name="psk", bufs=2, space="PSUM") as psk,
            tc.tile_pool(name="pskr", bufs=1, space="PSUM") as pskr,
        ):
            for p in range(2):
                tok = slice(p * HN, (p + 1) * HN)
                xb3 = []
                xb_q = (nc.sync, nc.scalar, nc.sync)
                for i in range(3):
                    t3 = pxb.tile([128, 4, HN], BF16, tag=f"xb{i}",
                                  name=f"xb{p}_{i}")
                    xb_q[i].dma_start(
                        out=t3,
                        in_=xbP[p * 128:(p + 1) * 128, i * 4 * HN:(i + 1) * 4 * HN],
                    )
                    xb3.append(t3)
                if p == 0:
                    # wv1 after p0's xb1 on scalar (queue transfers
                    # serialize; first ct1 use is ~27us in)
                    nc.scalar.dma_start(out=wvt[1], in_=WgT[:, 4608:9216])
                xb = [xb3[k // 4][:, k % 4, :] for k in range(KT)]
                # V projection: [token, vcol] tiles, 2 col tiles of 384.
                # PSUM accumulation is order-independent: consume xb tiles
                # in DMA-arrival order (xb0 sync, xb2 sync, xb1 scalar) so
                # the first chains start before all of x has landed
                KORD = [0, 1, 2, 3, 8, 9, 10, 11, 4, 5, 6, 7]
                for ct in range(2):
                    wv = wvt[ct]
                    for tt in range(HN // 128):
                        ps = psk.tile([128, 384], F32, tag="kps", name="vps")
                        for ki, k in enumerate(KORD):
                            nc.tensor.matmul(
                                out=ps, lhsT=xb[k][:, tt * 128:(tt + 1) * 128],
                                rhs=wv[:, k, :], start=(ki == 0),
                                stop=(ki == KT - 1),
                            )
                        vst = pvst.tile([128, 4, DH], BF16, tag="vst", name="vst")
                        nc.vector.tensor_copy(out=vst, in_=ps)
                        mt = p * (HN // 128) + tt
                        # alternate staging queues to halve queue latency
                        (nc.sync if tt % 2 == 0 else nc.scalar).dma_start(
                            out=Vr[:, mt, ct * 4:(ct + 1) * 4, 0:DH], in_=vst
                        )
                # K and Q projection, packed 128-row tiles + lagged rope
                for dst, si, nm in ((KTr, 0, "k"), (QTr, 1, "q")):
                    for rt in range(RT):
                        cs = 9216 + si * 9216 + rt * 1536
                        wk = pwkq.tile([128, KT, 128], BF16, tag="wkq",
                                       name=f"w{nm}{p}_{rt}")
                        nc.scalar.dma_start(out=wk, in_=WgT[:, cs:cs + 1536])
                        if p == 0 and si == 0 and rt == 0:
                            # trig behind the first K weights: its transfer
                            # must not race the startup-critical loads
                            # (DMA bandwidth is shared across queues);
                            # first rope use is ~45us in
                            nc.scalar.dma_start(out=cosR, in_=cosP)
                            nc.scalar.dma_start(out=sinR, in_=sinP)
                        ps = psk.tile([128, HN], F32, tag="kps", name="kps")
                        for k in range(KT):
                            mm512(ps, wk[:, k, :], xb[k], start=(k == 0),
                                  stop=(k == KT - 1))
                        t_sb = capture_rope(ps)
                        if rope_pending is not None:
                            emit_rope(*rope_pending)
                        rope_pending = (t_sb, rt, dst, tok)
            emit_rope(*rope_pending)

    # ---- attention + packed ho + interleaved output projection -----------
    span = ctx.enter_context(tc.tile_pool(name="span", bufs=1))
    hoP = span.tile([128, RT, N], BF16, tag="hoP")
    bias_sb = span.tile([128, DIM], F32, tag="bias")
    bout_bc = bass.AP(tensor=bout.tensor, offset=bout.offset,
                      ap=[[0, 128]] + [list(p) for p in bout.ap])
    nc.sync.dma_start(out=bias_sb, in_=bout_bc)
    wo = []
    for et in range(DIM // 512):
        wt = span.tile([128, RT, 512], BF16, tag=f"wo{et}", name=f"wo{et}")
        nc.sync.dma_start(out=wt, in_=WoT[:, et, :])
        wo.append(wt)
    with (
        tc.tile_pool(name="ex", bufs=4) as pex,
        tc.tile_pool(name="hur", bufs=3) as phur,
        tc.tile_pool(name="stn", bufs=2) as pstn,
        tc.tile_pool(name="rcd", bufs=1) as prcd,
        tc.tile_pool(name="rc", bufs=2) as prc,
        tc.tile_pool(name="osb", bufs=4) as posb,
        tc.tile_pool(name="rbd", bufs=2, space="DRAM") as prbd,
        tc.tile_pool(name="bcs", bufs=2) as pbcs,
        tc.tile_pool(name="pssc", bufs=3, space="PSUM") as pssc,
        tc.tile_pool(name="psho", bufs=2, space="PSUM") as psho,
    ):
        def normalize(h, qt, rcf, hu, hops_bc=False):
            """stage = hu * broadcast(rc); DMA-pack staged rows into hoP.

            The broadcast runs in the eps ring (idle during attention) so it
            never shrinks the sc ring's pipeline depth; the final flush uses
            the freed ho slot instead (eps+scps hold open pre-tail groups).
            """
            qsl = slice(qt * HN, (qt + 1) * HN)
            stn = pstn.tile([DH, HN], BF16, tag="stn", name="stn")
            if hops_bc:
                # final flush: PE is idle then, and the matmul broadcast is
                # lower-latency than the DRAM bounce; final_rc was converted
                # to f32r eagerly inside the last head's chain
                bc = psho.tile([DH, HN], F32, tag="hops", name="bcps",
                               bufs=1)
                mm512(bc, ones1, final_rc, start=True, stop=True)
                nc.vector.tensor_mul(out=stn, in0=hu[0:DH, :], in1=bc)
            else:
                # steady state: broadcast via DRAM bounce (stride-0 reads
                # are legal from DRAM) — zero PE work, latency hidden by
                # the one-head normalize lag
                rb = prbd.tile([1, HN], F32, tag="rbd", name="rbd")
                nc.sync.dma_start(out=rb, in_=rcf)
                rb_bc = bass.AP(tensor=rb.tensor, offset=rb.offset,
                                ap=[[0, DH]] + [list(p) for p in rb.ap[1:]])
                bc = pbcs.tile([DH, HN], F32, tag="bcs", name="bcs")
                nc.sync.dma_start(out=bc, in_=rb_bc)
                nc.vector.tensor_mul(out=stn, in0=hu[0:DH, :], in1=bc)
            for (rt, a, b, d0) in HEAD_SEGS[h]:
                nc.sync.dma_start(
                    out=hoP[a:b, rt, qsl], in_=stn[d0:d0 + (b - a), :]
                )

        out_qs = [nc.sync, nc.scalar]

        def emit_outproj(et, tt):
            """One [128-token, 512-col] output tile; own PSUM ring."""
            ps = pssc.tile([128, 512], F32, tag="scps", name="eps")
            for c in range(RT):
                nc.tensor.matmul(
                    out=ps, lhsT=hoP[:, c, tt * 128:(tt + 1) * 128],
                    rhs=wo[et][:, c, :], start=(c == 0), stop=(c == RT - 1),
                )
            osb = posb.tile([128, 512], F32, tag="osb", name="osb")
            nc.vector.tensor_add(
                out=osb, in0=ps, in1=bias_sb[:, et * 512:(et + 1) * 512]
            )
            out_qs[(et + tt) % 2].dma_start(
                out=out[tt * 128:(tt + 1) * 128, et * 512:(et + 1) * 512],
                in_=osb,
            )

        outq = []
        # mt slot -> chunk phase for spread-out drains
        DRAIN_MT = {5: 0, 6: 1, 7: 2, 10: 0, 11: 1, 12: 2, 13: 0, 14: 1,
                    15: 2}
        cur_drain = [None]

        def drain_chunk(phase):
            if phase == 0:
                if not outq:
                    cur_drain[0] = None
                    return
                et, tt = outq.pop(0)
                ps = pssc.tile([128, 512], F32, tag="scps", name="eps")
                cur_drain[0] = (et, tt, ps)
            if cur_drain[0] is None:
                return
            et, tt, ps = cur_drain[0]
            for c in (2 * phase, 2 * phase + 1):
                nc.tensor.matmul(
                    out=ps, lhsT=hoP[:, c, tt * 128:(tt + 1) * 128],
                    rhs=wo[et][:, c, :], start=(c == 0), stop=(c == RT - 1),
                )
            if phase == 2:
                osb = posb.tile([128, 512], F32, tag="osb", name="osb")
                nc.vector.tensor_add(
                    out=osb, in0=ps, in1=bias_sb[:, et * 512:(et + 1) * 512]
                )
                out_qs[(et + tt) % 2].dma_start(
                    out=out[tt * 128:(tt + 1) * 128,
                            et * 512:(et + 1) * 512],
                    in_=osb,
                )
                cur_drain[0] = None

        pending = None
        for qt in range(2):
            for h in range(HG):
                qsl = slice(qt * HN, (qt + 1) * HN)
                # single-buffered: freed right after the hu/dn reads below
                ho = psho.tile([DH + 1, HN], F32, tag="hops", name="hops",
                               bufs=1)

                def emit_ho(mt, ex):
                    mm512(ho, Vr[:, mt, h, :], ex,
                          start=(mt == 0), stop=(mt == NMT - 1))

                # ho matmuls lag one mt behind exp so the single-buffered
                # ho slot wait (prev head's hu read) never blocks the sc/exp
                # stream at head boundaries
                pend_ho = None
                for mt in range(NMT):
                    sc = pssc.tile([128, HN], F32, tag="scps", name="scps")
                    mm512(sc, KTr[:, h, mt * 128:(mt + 1) * 128],
                          QTr[:, h, qsl], start=True, stop=True)
                    ex = pex.tile([128, HN], BF16, tag="ex", name="ex")
                    nc.scalar.activation(out=ex, in_=sc, func=AF.Exp, scale=SCALE)
                    if pend_ho is not None:
                        emit_ho(*pend_ho)
                    pend_ho = (mt, ex)
                    if qt == 1 and h == 0 and mt == 2 and pending is not None:
                        # emit the lagged (qt0, h7) normalize early: its
                        # reciprocal is long done, and it unblocks qt0
                        # outproj drains already during this head
                        normalize(*pending)
                        pending = None
                    # drain finished-half outproj tiles into the PE slack
                    # left by the ACT-bound exp stream, spread as 2-matmul
                    # chunks so no single insertion outruns the sc ring
                    if qt == 1 and (h >= 1 or mt >= 5) and mt in DRAIN_MT:
                        drain_chunk(DRAIN_MT[mt])
                emit_ho(*pend_ho)
                # free ho with ONE f32 read (numerator rows + denom row);
                # recip input goes through a partition-0 copy (the custom
                # DVE recip mishandles nonzero base partitions)
                hu = phur.tile([DH + 1, HN], F32, tag="hur", name="hur")
                nc.vector.tensor_copy(out=hu, in_=ho)
                dn = prcd.tile([1, HN], F32, tag="dn", name="dn")
                nc.vector.tensor_copy(out=dn, in_=hu[DH:DH + 1, :])
                rcf = prc.tile([1, HN], F32, tag="rcf", name="rcf")
                nc.vector.reciprocal_approx_fast(out=rcf, in_=dn)
                if qt == 1 and h == HG - 1:
                    # eager f32r conversion so the tail's broadcast matmul
                    # isn't gated on one more serial DVE op
                    final_rc = prc.tile([1, HN], F32R, tag="rcf2",
                                        name="rcf2", bufs=1)
                    nc.vector.tensor_copy(out=final_rc, in_=rcf)
                if pending is not None:
                    normalize(*pending)
                pending = (h, qt, rcf, hu)
            if qt == 0:
                outq = [(et, tt) for et in range(DIM // 512)
                        for tt in range(NMT // 2)]
        # tail: pre-start four tiles on contraction rows 0..4 (heads 0..6,
        # already normalized) so the PE stays busy through the final
        # normalize's reciprocal chain; then finish them and run the rest
        tail = [(et, tt) for tt in range(NMT // 2, NMT)
                for et in range(DIM // 512)]
        NPRE = 6
        pre_aps = []
        for _ in range(3):
            # each idle scps slab provides two independent half-bank groups
            slab = pssc.tile([128, HN], F32, tag="scps", name="pre_slab")
            pre_aps.append(slab[:, 0:512])
            pre_aps.append(slab[:, 512:1024])
        pre = []
        for (et, tt), ps in zip(tail[:NPRE], pre_aps):
            for c in range(RT - 1):
                nc.tensor.matmul(
                    out=ps, lhsT=hoP[:, c, tt * 128:(tt + 1) * 128],
                    rhs=wo[et][:, c, :], start=(c == 0), stop=False,
                )
            pre.append(ps)
        normalize(*pending, hops_bc=True)
        for (et, tt), ps in zip(tail[:NPRE], pre):
            nc.tensor.matmul(
                out=ps, lhsT=hoP[:, RT - 1, tt * 128:(tt + 1) * 128],
                rhs=wo[et][:, RT - 1, :], start=False, stop=True,
            )
            osb = posb.tile([128, 512], F32, tag="osb", name="osb")
            nc.vector.tensor_add(
                out=osb, in0=ps, in1=bias_sb[:, et * 512:(et + 1) * 512]
            )
            out_qs[(et + tt) % 2].dma_start(
                out=out[tt * 128:(tt + 1) * 128, et * 512:(et + 1) * 512],
                in_=osb,
            )
        for et, tt in outq:
            emit_outproj(et, tt)
        for et, tt in tail[NPRE:]:
            emit_outproj(et, tt)


def build():
    from contextlib import ExitStack

    nc = bacc.Bacc("TRN2", target_bir_lowering=False, debug=False)
    xbP = nc.declare_dram_parameter("xbP", [2 * 128, KT * HN], BF16, isOutput=False)
    cosP = nc.declare_dram_parameter("cosR", [128, 3, N], BF16, isOutput=False)
    sinP = nc.declare_dram_parameter("sinR", [128, 3, N], BF16, isOutput=False)
    ropePT = nc.declare_dram_parameter("ropePT", [128, 3 * 128], BF16, isOutput=False)
    WgT = nc.declare_dram_parameter("WgT", [128, 27648], BF16, isOutput=False)
    WoT = nc.declare_dram_parameter("WoT", [128, 3, RT * 512], BF16, isOutput=False)
    bout_p = nc.declare_dram_parameter("bout", [DIM], F32, isOutput=False)
    out = nc.declare_dram_parameter("out", [N, DIM], F32, isOutput=True)
    io = tuple(t[:] for t in (xbP, cosP, sinP, ropePT, WgT, WoT, bout_p, out))
    with ExitStack() as ctx:
        tc = ctx.enter_context(tile.TileContext(nc))
        _emit(ctx, tc, io)
    nc.finalize()
    return nc


def make_in_maps(x, f1, f2, f3, Wqkv, Wout, bout):
    x = np.asarray(x, np.float32)
    fcat = np.concatenate(
        [np.asarray(f1, np.float32), np.asarray(f2, np.float32),
         np.asarray(f3, np.float32)], axis=1,
    )  # [N, DH]
    idx = (128 * np.arange(3)[:, None] + np.arange(128)[None, :]) % 96  # [3, 128]
    fR = fcat.T[idx].transpose(1, 0, 2)  # [128, 3, N]
    cosR_np = np.ascontiguousarray(np.cos(fR)).astype(IN_NP)
    sinR_np = np.ascontiguousarray(np.sin(fR)).astype(IN_NP)
    PT = _build_rope_pt_packed().astype(IN_NP)
    Wqkv = np.asarray(Wqkv, np.float32)
    Wout = np.asarray(Wout, np.float32)
    bout = np.ascontiguousarray(np.asarray(bout, np.float32))
    zeros_bias = np.zeros_like(bout)
    # x pre-tiled: [2, 128, KT, HN] -> [256, KT*HN]; per-partition contiguous
    xP = []
    for b in range(B):
        xb = x[b].T.reshape(KT, 128, 2, HN)           # [k, p, half, n]
        xb = xb.transpose(2, 1, 0, 3).reshape(2 * 128, KT * HN)
        xP.append(np.ascontiguousarray(xb).astype(IN_NP))
    WgT_g, WoT_g = [], []
    for g in range(2):
        cols = np.concatenate(
            [Wqkv[:, g * G:(g + 1) * G],
             Wqkv[:, H * DH + g * G:H * DH + (g + 1) * G],
             Wqkv[:, 2 * H * DH + g * G:2 * H * DH + (g + 1) * G]], axis=1,
        )  # [DIM, 3G]: [Q | K | V] for this group
        W3 = cols.reshape(KT, 128, 3 * G)
        blocks = []
        for ct in range(2):  # V tiles
            cs = 2 * G + ct * 384
            blocks.append(W3[:, :, cs:cs + 384].transpose(1, 0, 2).reshape(128, KT * 384))
        for si, base in ((0, G), (1, 0)):  # K tiles then Q tiles
            for rt in range(RT):
                cs = base + rt * 128
                blocks.append(W3[:, :, cs:cs + 128].transpose(1, 0, 2).reshape(128, KT * 128))
        WgT_g.append(np.ascontiguousarray(np.concatenate(blocks, axis=1)).astype(IN_NP))
        WoG_ = Wout[g * G:(g + 1) * G, :]
        Wo3 = WoG_.reshape(RT, 128, DIM)
        ets = [Wo3[:, :, et * 512:(et + 1) * 512].transpose(1, 0, 2).reshape(128, 1, RT * 512)
               for et in range(3)]
        WoT_g.append(np.ascontiguousarray(np.concatenate(ets, axis=1)).astype(IN_NP))
    in_maps = []
    for c in range(8):
        b, g = divmod(c, 2)
        in_maps.append(dict(
            xbP=xP[b], cosR=cosR_np, sinR=sinR_np, ropePT=PT,
            WgT=WgT_g[g], WoT=WoT_g[g],
            bout=(bout if g == 0 else zeros_bias),
        ))
    return in_maps


_NC_CACHE = None


def kernel(x, f1, f2, f3, Wqkv, Wout, bout, _trace=False):
    global _NC_CACHE
    if _NC_CACHE is None:
        _NC_CACHE = build()
    nc = _NC_CACHE
    in_maps = make_in_maps(x, f1, f2, f3, Wqkv, Wout, bout)
    res = run_bass_kernel_spmd(nc, in_maps, list(range(8)), trace=_trace)
    out = np.empty((B, N, DIM), np.float32)
    for b in range(B):
        out[b] = res.results[2 * b]["out"]
        out[b] += res.results[2 * b + 1]["out"]
    if _trace:
        return out, res
    return out



# revision 78
# speedup vs baseline: 1.0095x; 1.0017x over previous
"""Trainium2 Bass kernel: 3D-RoPE multi-head attention (B=4,N=2048,DIM=1536,H=16,DH=96).

Sharding: 8 cores = (batch b = c//2) x (head-group g = c%2, 8 heads each).
Each core:
  - projects Q,K,V for its 8 heads over all 2048 tokens (tensor parallel:
    Wqkv column-split, Wout row-split). K/Q projected in PACKED 128-row
    tiles of the [768, N] K^T/Q^T matrix (full PE-array utilization), RoPE
    applied per packed tile (3 distinct 128x128 sign-permutations,
    host-precomputed replicated cos/sin), then DMA-unpacked into per-head
    [96, N] tiles.
  - attention per head (softmax over keys, no max-subtraction), query-half
    outer loop; output-projection tiles for the finished query half are
    interleaved into the next half's head loop to fill the PE slack left
    by the ACT-bound exp stream (outproj PSUM shares the ho pool ring).
  - normalized head outputs DMA-packed into [128, 6, N] tiles; partial
    output projection contracts 6x128 rows of Wout.
Host gather: out[b] = core(2b) + core(2b+1) partial sums (bias on g=0).
All matmul inputs bf16, fp32 PSUM accumulation, no DRAM spills.
"""

import sys

if "/opt/trn_rl_repo" not in sys.path:
    sys.path.insert(0, "/opt/trn_rl_repo")

import numpy as np

import concourse.bass as bass
import concourse.mybir as mybir
import concourse.tile as tile
from concourse import bacc
from concourse.bass_utils import run_bass_kernel_spmd

B, N, DIM, H, DH = 4, 2048, 1536, 16, 96
HG = 8            # heads per core
G = HG * DH       # 768 rows/cols per group
RT = G // 128     # 6 packed row tiles
KT = DIM // 128   # 12 contraction tiles
HN = N // 2       # token half (projection passes) / query chunk
NMT = N // 128    # 16 key tiles
SCALE = DH ** -0.5
F32 = mybir.dt.float32
F32R = mybir.dt.float32r
BF16 = mybir.dt.bfloat16
F16 = mybir.dt.float16
import ml_dtypes
IN_NP = ml_dtypes.bfloat16
AF = mybir.ActivationFunctionType

# packed-tile segment maps (128-row tile j of a [384,*] block, period 3)
# replica rows: row r of tile j = master row (128*j + r) % 96
REP_SEGS = {
    0: [(0, 96, 0), (96, 128, 0)],
    1: [(0, 64, 32), (64, 128, 0)],
    2: [(0, 32, 64), (32, 128, 0)],
}
# per packed tile rt: (head, tile_lo, tile_hi, head_row_lo)
def _tile_segs(rt):
    segs = []
    r = 128 * rt
    while r < 128 * rt + 128:
        h = r // 96
        end = min(128 * rt + 128, (h + 1) * 96)
        segs.append((h, r - 128 * rt, end - 128 * rt, r % 96))
        r = end
    return segs
TILE_SEGS = {rt: _tile_segs(rt) for rt in range(RT)}
# per head: list of (rt, tile_lo, tile_hi, head_row_lo)
HEAD_SEGS = {h: [] for h in range(HG)}
for rt in range(RT):
    for h, a, b, d0 in TILE_SEGS[rt]:
        HEAD_SEGS[h].append((rt, a, b, d0))


def _build_rope_pt_packed() -> np.ndarray:
    """lhsT tiles for rotate_half on packed 128-row layout: A[:, j*128:(j+1)*128]
    = P_j^T where rot_packed = P_j @ t_packed for tile j (j = rt % 3)."""
    A = np.zeros((128, 3 * 128), np.float32)
    for j in range(3):
        for r in range(128):
            Rg = 128 * j + r
            d = Rg % 96
            c, a = d // 32, d % 32
            sign = -1.0 if a < 16 else 1.0
            dq = 32 * c + (a + 16) % 32
            qg = (Rg // 96) * 96 + dq
            ql = qg - 128 * j
            # P_j[r, ql] = sign; lhsT[k, i] = P_j[i, k]
            A[ql, j * 128 + r] = sign
    return np.ascontiguousarray(A)


def _emit(ctx, tc, io):
    nc = tc.nc
    xbP, cosP, sinP, ropePT, WgT, WoT, bout, out = io

    def mm512(out_ap, lhsT, rhs, start, stop, width=HN):
        for c0 in range(0, width, 512):
            nc.tensor.matmul(
                out=out_ap[:, c0:c0 + 512], lhsT=lhsT,
                rhs=rhs[:, c0:c0 + 512], start=start, stop=stop,
            )

    persist = ctx.enter_context(tc.tile_pool(name="persist", bufs=1))

    # ---- constants ------------------------------------------------------
    ropeP_sb = persist.tile([128, 3, 128], BF16, tag="ropeP")
    nc.sync.dma_start(out=ropeP_sb, in_=ropePT.rearrange("p (j c) -> p j c", j=3))
    ones1f = persist.tile([1, DH], F32, tag="ones1f")
    nc.vector.memset(ones1f, 1.0)
    ones1 = persist.tile([1, DH], F32R, tag="ones1")
    nc.scalar.copy(out=ones1, in_=ones1f)

    # resident tensors
    KTr = persist.tile([DH, HG, N], BF16, tag="KTr")
    QTr = persist.tile([DH, HG, N], BF16, tag="QTr")
    Vr = persist.tile([128, NMT, HG, DH + 1], BF16, tag="Vr")
    nc.vector.memset(Vr[:, :, :, DH:DH + 1], 1.0)

    # ---- projections (two token-half passes, shared xb tiles) ------------
    # WgT columns: [V ct0 | V ct1 | K rt0..5 | Q rt0..5], each tile k-major
    # and per-partition contiguous. xbP rows: [p*128 + partition].
    with tc.tile_pool(name="wv", bufs=1) as pwv:
        # V weights are token-independent: both column tiles load once for
        # both passes; wv0 leads the scalar queue (first-matmul critical
        # path), wv1 follows p0's xb1
        wvt = [pwv.tile([128, KT, 384], BF16, tag=f"wv{ct}", name=f"wv{ct}")
               for ct in range(2)]
        nc.scalar.dma_start(out=wvt[0], in_=WgT[:, 0:4608])
        # host-precomputed replicated cos/sin for packed rope [128, 3, N];
        # projection-scoped so attention inherits the 24KB
        cosR = pwv.tile([128, 3, N], BF16, tag="cosR")
        sinR = pwv.tile([128, 3, N], BF16, tag="sinR")

        rope_pending = None

        def capture_rope(ps):
            """Free the projection PSUM group immediately: the copy runs
            ahead of the lagged rope's muls on the in-order DVE queue, so
            bank release (and the proj->attention handoff) isn't serialized
            behind trig work."""
            t_sb = prp.tile([128, HN], BF16, tag="rope_t", name="rope_t")
            nc.vector.tensor_copy(out=t_sb, in_=ps)
            return t_sb

        def emit_rope(t_sb, rt, dst, tok):
            """Packed rope on a captured projection tile -> dst per-head."""
            j = rt % 3
            rot = pskr.tile([128, HN], F32, tag="rope_rot", name="rope_rot")
            mm512(rot, ropeP_sb[:, j, :], t_sb, start=True, stop=True)
            u = prp.tile([128, HN], BF16, tag="rope_u", name="rope_u")
            nc.vector.tensor_mul(out=u, in0=t_sb, in1=cosR[:, j, tok])
            nc.vector.tensor_mul(out=rot, in0=rot, in1=sinR[:, j, tok])
            st = prp.tile([128, HN], BF16, tag="rope_st", name="rope_st")
            nc.vector.tensor_add(out=st, in0=u, in1=rot)
            for (h, a, b, d0) in TILE_SEGS[rt]:
                nc.sync.dma_start(
                    out=dst[d0:d0 + (b - a), h, tok], in_=st[a:b, :]
                )

        with (
            tc.tile_pool(name="xb", bufs=2) as pxb,
            tc.tile_pool(name="wkq", bufs=3) as pwkq,
            tc.tile_pool(name="vst", bufs=8) as pvst,
            tc.tile_pool(name="rope", bufs=2) as prp,
            tc.tile_pool(name="psk", bufs=2, space="PSUM") as psk,
            tc.tile_pool(name="pskr", bufs=1, space="PSUM") as pskr,
        ):
            for p in range(2):
                tok = slice(p * HN, (p + 1) * HN)
                xb3 = []
                xb_q = (nc.sync, nc.scalar, nc.sync)
                for i in range(3):
                    t3 = pxb.tile([128, 4, HN], BF16, tag=f"xb{i}",
                                  name=f"xb{p}_{i}")
                    xb_q[i].dma_start(
                        out=t3,
                        in_=xbP[p * 128:(p + 1) * 128, i * 4 * HN:(i + 1) * 4 * HN],
                    )
                    xb3.append(t3)
                # (wv1 is emitted later, behind the first K weights, to
                # keep its transfer out of the startup-critical DMA window)
                xb = [xb3[k // 4][:, k % 4, :] for k in range(KT)]
                # V projection: [token, vcol] tiles, 2 col tiles of 384.
                # PSUM accumulation is order-independent: consume xb tiles
                # in DMA-arrival order (xb0 sync, xb2 sync, xb1 scalar) so
                # the first chains start before all of x has landed
                KORD = [0, 1, 2, 3, 8, 9, 10, 11, 4, 5, 6, 7]
                for ct in range(2):
                    wv = wvt[ct]
                    for tt in range(HN // 128):
                        ps = psk.tile([128, 384], F32, tag="kps", name="vps")
                        for ki, k in enumerate(KORD):
                            nc.tensor.matmul(
                                out=ps, lhsT=xb[k][:, tt * 128:(tt + 1) * 128],
                                rhs=wv[:, k, :], start=(ki == 0),
                                stop=(ki == KT - 1),
                            )
                        vst = pvst.tile([128, 4, DH], BF16, tag="vst", name="vst")
                        nc.vector.tensor_copy(out=vst, in_=ps)
                        mt = p * (HN // 128) + tt
                        # alternate staging queues to halve queue latency
                        (nc.sync if tt % 2 == 0 else nc.scalar).dma_start(
                            out=Vr[:, mt, ct * 4:(ct + 1) * 4, 0:DH], in_=vst
                        )
                # K and Q projection, packed 128-row tiles + lagged rope
                for dst, si, nm in ((KTr, 0, "k"), (QTr, 1, "q")):
                    for rt in range(RT):
                        cs = 9216 + si * 9216 + rt * 1536
                        wk = pwkq.tile([128, KT, 128], BF16, tag="wkq",
                                       name=f"w{nm}{p}_{rt}")
                        nc.scalar.dma_start(out=wk, in_=WgT[:, cs:cs + 1536])
                        if p == 0 and si == 0 and rt == 0:
                            # wv1 and trig behind the first K weights: their
                            # transfers must not race the startup-critical
                            # loads (DMA bandwidth is shared across queues);
                            # wv1 is used ~27us in, first rope ~45us in
                            nc.scalar.dma_start(out=wvt[1],
                                                in_=WgT[:, 4608:9216])
                            nc.scalar.dma_start(out=cosR, in_=cosP)
                            nc.scalar.dma_start(out=sinR, in_=sinP)
                        ps = psk.tile([128, HN], F32, tag="kps", name="kps")
                        for k in range(KT):
                            mm512(ps, wk[:, k, :], xb[k], start=(k == 0),
                                  stop=(k == KT - 1))
                        t_sb = capture_rope(ps)
                        if rope_pending is not None:
                            emit_rope(*rope_pending)
                        rope_pending = (t_sb, rt, dst, tok)
            emit_rope(*rope_pending)

    # ---- attention + packed ho + interleaved output projection -----------
    span = ctx.enter_context(tc.tile_pool(name="span", bufs=1))
    hoP = span.tile([128, RT, N], BF16, tag="hoP")
    bias_sb = span.tile([128, DIM], F32, tag="bias")
    bout_bc = bass.AP(tensor=bout.tensor, offset=bout.offset,
                      ap=[[0, 128]] + [list(p) for p in bout.ap])
    nc.sync.dma_start(out=bias_sb, in_=bout_bc)
    wo = []
    for et in range(DIM // 512):
        wt = span.tile([128, RT, 512], BF16, tag=f"wo{et}", name=f"wo{et}")
        nc.sync.dma_start(out=wt, in_=WoT[:, et, :])
        wo.append(wt)
    with (
        tc.tile_pool(name="ex", bufs=4) as pex,
        tc.tile_pool(name="hur", bufs=3) as phur,
        tc.tile_pool(name="stn", bufs=2) as pstn,
        tc.tile_pool(name="rcd", bufs=1) as prcd,
        tc.tile_pool(name="rc", bufs=2) as prc,
        tc.tile_pool(name="osb", bufs=4) as posb,
        tc.tile_pool(name="rbd", bufs=2, space="DRAM") as prbd,
        tc.tile_pool(name="bcs", bufs=2) as pbcs,
        tc.tile_pool(name="pssc", bufs=3, space="PSUM") as pssc,
        tc.tile_pool(name="psho", bufs=2, space="PSUM") as psho,
    ):
        def normalize(h, qt, rcf, hu, hops_bc=False):
            """stage = hu * broadcast(rc); DMA-pack staged rows into hoP.

            The broadcast runs in the eps ring (idle during attention) so it
            never shrinks the sc ring's pipeline depth; the final flush uses
            the freed ho slot instead (eps+scps hold open pre-tail groups).
            """
            qsl = slice(qt * HN, (qt + 1) * HN)
            stn = pstn.tile([DH, HN], BF16, tag="stn", name="stn")
            if hops_bc:
                # final flush: PE is idle then, and the matmul broadcast is
                # lower-latency than the DRAM bounce; final_rc was converted
                # to f32r eagerly inside the last head's chain
                bc = psho.tile([DH, HN], F32, tag="hops", name="bcps",
                               bufs=1)
                mm512(bc, ones1, final_rc, start=True, stop=True)
                nc.vector.tensor_mul(out=stn, in0=hu[0:DH, :], in1=bc)
            else:
                # steady state: broadcast via DRAM bounce (stride-0 reads
                # are legal from DRAM) — zero PE work, latency hidden by
                # the one-head normalize lag
                rb = prbd.tile([1, HN], F32, tag="rbd", name="rbd")
                nc.sync.dma_start(out=rb, in_=rcf)
                rb_bc = bass.AP(tensor=rb.tensor, offset=rb.offset,
                                ap=[[0, DH]] + [list(p) for p in rb.ap[1:]])
                bc = pbcs.tile([DH, HN], F32, tag="bcs", name="bcs")
                nc.sync.dma_start(out=bc, in_=rb_bc)
                nc.vector.tensor_mul(out=stn, in0=hu[0:DH, :], in1=bc)
            for (rt, a, b, d0) in HEAD_SEGS[h]:
                nc.sync.dma_start(
                    out=hoP[a:b, rt, qsl], in_=stn[d0:d0 + (b - a), :]
                )

        out_qs = [nc.sync, nc.scalar]

        def emit_outproj(et, tt):
            """One [128-token, 512-col] output tile; own PSUM ring."""
            ps = pssc.tile([128, 512], F32, tag="scps", name="eps")
            for c in range(RT):
                nc.tensor.matmul(
                    out=ps, lhsT=hoP[:, c, tt * 128:(tt + 1) * 128],
                    rhs=wo[et][:, c, :], start=(c == 0), stop=(c == RT - 1),
                )
            osb = posb.tile([128, 512], F32, tag="osb", name="osb")
            nc.vector.tensor_add(
                out=osb, in0=ps, in1=bias_sb[:, et * 512:(et + 1) * 512]
            )
            out_qs[(et + tt) % 2].dma_start(
                out=out[tt * 128:(tt + 1) * 128, et * 512:(et + 1) * 512],
                in_=osb,
            )

        outq = []
        # mt slot -> chunk phase for spread-out drains
        DRAIN_MT = {5: 0, 6: 1, 7: 2, 10: 0, 11: 1, 12: 2, 13: 0, 14: 1,
                    15: 2}
        cur_drain = [None]

        def drain_chunk(phase):
            if phase == 0:
                if not outq:
                    cur_drain[0] = None
                    return
                et, tt = outq.pop(0)
                ps = pssc.tile([128, 512], F32, tag="scps", name="eps")
                cur_drain[0] = (et, tt, ps)
            if cur_drain[0] is None:
                return
            et, tt, ps = cur_drain[0]
            for c in (2 * phase, 2 * phase + 1):
                nc.tensor.matmul(
                    out=ps, lhsT=hoP[:, c, tt * 128:(tt + 1) * 128],
                    rhs=wo[et][:, c, :], start=(c == 0), stop=(c == RT - 1),
                )
            if phase == 2:
                osb = posb.tile([128, 512], F32, tag="osb", name="osb")
                nc.vector.tensor_add(
                    out=osb, in0=ps, in1=bias_sb[:, et * 512:(et + 1) * 512]
                )
                out_qs[(et + tt) % 2].dma_start(
                    out=out[tt * 128:(tt + 1) * 128,
                            et * 512:(et + 1) * 512],
                    in_=osb,
                )
                cur_drain[0] = None

        pending = None
        for qt in range(2):
            for h in range(HG):
                qsl = slice(qt * HN, (qt + 1) * HN)
                # single-buffered: freed right after the hu/dn reads below
                ho = psho.tile([DH + 1, HN], F32, tag="hops", name="hops",
                               bufs=1)

                def emit_ho(mt, ex):
                    mm512(ho, Vr[:, mt, h, :], ex,
                          start=(mt == 0), stop=(mt == NMT - 1))

                # ho matmuls lag one mt behind exp so the single-buffered
                # ho slot wait (prev head's hu read) never blocks the sc/exp
                # stream at head boundaries
                pend_ho = None
                for mt in range(NMT):
                    sc = pssc.tile([128, HN], F32, tag="scps", name="scps")
                    mm512(sc, KTr[:, h, mt * 128:(mt + 1) * 128],
                          QTr[:, h, qsl], start=True, stop=True)
                    ex = pex.tile([128, HN], BF16, tag="ex", name="ex")
                    nc.scalar.activation(out=ex, in_=sc, func=AF.Exp, scale=SCALE)
                    if pend_ho is not None:
                        emit_ho(*pend_ho)
                    pend_ho = (mt, ex)
                    if qt == 1 and h == 0 and mt == 2 and pending is not None:
                        # emit the lagged (qt0, h7) normalize early: its
                        # reciprocal is long done, and it unblocks qt0
                        # outproj drains already during this head
                        normalize(*pending)
                        pending = None
                    # drain finished-half outproj tiles into the PE slack
                    # left by the ACT-bound exp stream, spread as 2-matmul
                    # chunks so no single insertion outruns the sc ring
                    if qt == 1 and (h >= 1 or mt >= 5) and mt in DRAIN_MT:
                        drain_chunk(DRAIN_MT[mt])
                emit_ho(*pend_ho)
                # free ho with ONE f32 read (numerator rows + denom row);
                # recip input goes through a partition-0 copy (the custom
                # DVE recip mishandles nonzero base partitions)
                hu = phur.tile([DH + 1, HN], F32, tag="hur", name="hur")
                nc.vector.tensor_copy(out=hu, in_=ho)
                dn = prcd.tile([1, HN], F32, tag="dn", name="dn")
                nc.vector.tensor_copy(out=dn, in_=hu[DH:DH + 1, :])
                rcf = prc.tile([1, HN], F32, tag="rcf", name="rcf")
                nc.vector.reciprocal_approx_fast(out=rcf, in_=dn)
                if qt == 1 and h == HG - 1:
                    # eager f32r conversion so the tail's broadcast matmul
                    # isn't gated on one more serial DVE op
                    final_rc = prc.tile([1, HN], F32R, tag="rcf2",
                                        name="rcf2", bufs=1)
                    nc.vector.tensor_copy(out=final_rc, in_=rcf)
                if pending is not None:
                    normalize(*pending)
                pending = (h, qt, rcf, hu)
            if qt == 0:
                outq = [(et, tt) for et in range(DIM // 512)
                        for tt in range(NMT // 2)]
        # tail: pre-start four tiles on contraction rows 0..4 (heads 0..6,
        # already normalized) so the PE stays busy through the final
        # normalize's reciprocal chain; then finish them and run the rest
        tail = [(et, tt) for tt in range(NMT // 2, NMT)
                for et in range(DIM // 512)]
        NPRE = 6
        pre_aps = []
        for _ in range(3):
            # each idle scps slab provides two independent half-bank groups
            slab = pssc.tile([128, HN], F32, tag="scps", name="pre_slab")
            pre_aps.append(slab[:, 0:512])
            pre_aps.append(slab[:, 512:1024])
        pre = []
        for (et, tt), ps in zip(tail[:NPRE], pre_aps):
            for c in range(RT - 1):
                nc.tensor.matmul(
                    out=ps, lhsT=hoP[:, c, tt * 128:(tt + 1) * 128],
                    rhs=wo[et][:, c, :], start=(c == 0), stop=False,
                )
            pre.append(ps)
        normalize(*pending, hops_bc=True)
        for (et, tt), ps in zip(tail[:NPRE], pre):
            nc.tensor.matmul(
                out=ps, lhsT=hoP[:, RT - 1, tt * 128:(tt + 1) * 128],
                rhs=wo[et][:, RT - 1, :], start=False, stop=True,
            )
            osb = posb.tile([128, 512], F32, tag="osb", name="osb")
            nc.vector.tensor_add(
                out=osb, in0=ps, in1=bias_sb[:, et * 512:(et + 1) * 512]
            )
            out_qs[(et + tt) % 2].dma_start(
                out=out[tt * 128:(tt + 1) * 128, et * 512:(et + 1) * 512],
                in_=osb,
            )
        for et, tt in outq:
            emit_outproj(et, tt)
        for et, tt in tail[NPRE:]:
            emit_outproj(et, tt)


def build():
    from contextlib import ExitStack

    nc = bacc.Bacc("TRN2", target_bir_lowering=False, debug=False)
    xbP = nc.declare_dram_parameter("xbP", [2 * 128, KT * HN], BF16, isOutput=False)
    cosP = nc.declare_dram_parameter("cosR", [128, 3, N], BF16, isOutput=False)
    sinP = nc.declare_dram_parameter("sinR", [128, 3, N], BF16, isOutput=False)
    ropePT = nc.declare_dram_parameter("ropePT", [128, 3 * 128], BF16, isOutput=False)
    WgT = nc.declare_dram_parameter("WgT", [128, 27648], BF16, isOutput=False)
    WoT = nc.declare_dram_parameter("WoT", [128, 3, RT * 512], BF16, isOutput=False)
    bout_p = nc.declare_dram_parameter("bout", [DIM], F32, isOutput=False)
    out = nc.declare_dram_parameter("out", [N, DIM], F32, isOutput=True)
    io = tuple(t[:] for t in (xbP, cosP, sinP, ropePT, WgT, WoT, bout_p, out))
    with ExitStack() as ctx:
        tc = ctx.enter_context(tile.TileContext(nc))
        _emit(ctx, tc, io)
    nc.finalize()
    return nc


def make_in_maps(x, f1, f2, f3, Wqkv, Wout, bout):
    x = np.asarray(x, np.float32)
    fcat = np.concatenate(
        [np.asarray(f1, np.float32), np.asarray(f2, np.float32),
         np.asarray(f3, np.float32)], axis=1,
    )  # [N, DH]
    idx = (128 * np.arange(3)[:, None] + np.arange(128)[None, :]) % 96  # [3, 128]
    fR = fcat.T[idx].transpose(1, 0, 2)  # [128, 3, N]
    cosR_np = np.ascontiguousarray(np.cos(fR)).astype(IN_NP)
    sinR_np = np.ascontiguousarray(np.sin(fR)).astype(IN_NP)
    PT = _build_rope_pt_packed().astype(IN_NP)
    Wqkv = np.asarray(Wqkv, np.float32)
    Wout = np.asarray(Wout, np.float32)
    bout = np.ascontiguousarray(np.asarray(bout, np.float32))
    zeros_bias = np.zeros_like(bout)
    # x pre-tiled: [2, 128, KT, HN] -> [256, KT*HN]; per-partition contiguous
    xP = []
    for b in range(B):
        xb = x[b].T.reshape(KT, 128, 2, HN)           # [k, p, half, n]
        xb = xb.transpose(2, 1, 0, 3).reshape(2 * 128, KT * HN)
        xP.append(np.ascontiguousarray(xb).astype(IN_NP))
    WgT_g, WoT_g = [], []
    for g in range(2):
        cols = np.concatenate(
            [Wqkv[:, g * G:(g + 1) * G],
             Wqkv[:, H * DH + g * G:H * DH + (g + 1) * G],
             Wqkv[:, 2 * H * DH + g * G:2 * H * DH + (g + 1) * G]], axis=1,
        )  # [DIM, 3G]: [Q | K | V] for this group
        W3 = cols.reshape(KT, 128, 3 * G)
        blocks = []
        for ct in range(2):  # V tiles
            cs = 2 * G + ct * 384
            blocks.append(W3[:, :, cs:cs + 384].transpose(1, 0, 2).reshape(128, KT * 384))
        for si, base in ((0, G), (1, 0)):  # K tiles then Q tiles
            for rt in range(RT):
                cs = base + rt * 128
                blocks.append(W3[:, :, cs:cs + 128].transpose(1, 0, 2).reshape(128, KT * 128))
        WgT_g.append(np.ascontiguousarray(np.concatenate(blocks, axis=1)).astype(IN_NP))
        WoG_ = Wout[g * G:(g + 1) * G, :]
        Wo3 = WoG_.reshape(RT, 128, DIM)
        ets = [Wo3[:, :, et * 512:(et + 1) * 512].transpose(1, 0, 2).reshape(128, 1, RT * 512)
               for et in range(3)]
        WoT_g.append(np.ascontiguousarray(np.concatenate(ets, axis=1)).astype(IN_NP))
    in_maps = []
    for c in range(8):
        b, g = divmod(c, 2)
        in_maps.append(dict(
            xbP=xP[b], cosR=cosR_np, sinR=sinR_np, ropePT=PT,
            WgT=WgT_g[g], WoT=WoT_g[g],
            bout=(bout if g == 0 else zeros_bias),
        ))
    return in_maps


_NC_CACHE = None


def kernel(x, f1, f2, f3, Wqkv, Wout, bout, _trace=False):
    global _NC_CACHE
    if _NC_CACHE is None:
        _NC_CACHE = build()
    nc = _NC_CACHE
    in_maps = make_in_maps(x, f1, f2, f3, Wqkv, Wout, bout)
    res = run_bass_kernel_spmd(nc, in_maps, list(range(8)), trace=_trace)
    out = np.empty((B, N, DIM), np.float32)
    for b in range(B):
        out[b] = res.results[2 * b]["out"]
        out[b] += res.results[2 * b + 1]["out"]
    if _trace:
        return out, res
    return out

